# revision 15
# baseline (speedup 1.0000x reference)
"""Trainium2 Bass kernel for nn_Bond2AtomLayer (GNN message passing).

Strategy (8-core SPMD, dst-node partitioned):
- Host: sort edges by dst, partition nodes into 8 ranges of 6250; each core
  owns the edges whose dst falls in its range. Within a core, edges are
  grouped into 128-node "blocks" (49 per core), each block's edges padded to
  a whole number of 128-edge tiles. Blocks are assigned to fixed program
  positions with a global per-position tile count (max over cores) so all
  8 cores run one identical program.
- Host pre-gathers node_emb[src], node_emb[dst] and bond rows into per-core
  edge-order streams, transposed to [feat, edge] (bf16) so the device needs
  no gather at all — k/q/v are computed per 128-edge tile by PE matmuls
  against the small weight matrices.
- Edge softmax: att[e,h] = sum_d k[e,hd]*q[e,hd] (DVE mult + grouped reduce),
  logits = att/4 + ba*W_dis (1/4 folded into Wq), p = exp(logits) without
  max-subtraction (logits are O(1); softmax is shift-invariant so this
  matches the reference numerically).
- Scatter-sum: per tile a one-hot S[e,n] (bf16, built by GPSIMD local_scatter)
  and one PE matmul accumulates [wv | p] into PSUM per 128-node block;
  ft = wsum / s, then beta-gating, LN, FFN, LN on-chip (node-major, batched).
"""
import sys

sys.path.insert(0, "/opt/trn_rl_repo")

import numpy as np
import ml_dtypes
from contextlib import ExitStack

import concourse.bass as bass
import concourse.tile as tile
from concourse import bacc, mybir
from concourse.bass_utils import run_bass_kernel_spmd

BF16 = ml_dtypes.bfloat16

N_NODES = 50000
N_EDGES = 800000
D = 128
H = 8
DH = 16
D_FF = 256
P = 128
NCORES = 8
NPC = N_NODES // NCORES        # 6250 nodes per core
NBLK = (NPC + P - 1) // P      # 49 blocks per core (last has 106 nodes)
NPAD = NBLK * P                # 6272
EPS = 1e-5
SEPS = 1e-30                   # guard for 1/s on isolated nodes
W136 = D + H                   # 136: [wv | p] scatter payload width

_CACHE = {}


# --------------------------------------------------------------------------
# host-side scheduling
# --------------------------------------------------------------------------

def _schedule(dst):
    """Partition edges by dst; build per-core block schedules.

    Returns:
      T_sched: list of per-position tile counts (same for all cores)
      chunks:  list of chunk widths per position (even, <=4 each)
      per_core: list of dicts with keys:
        edge_perm [TT*128] int64 (index into full edge list; -1 = pad)
        dst_local [TT*128] int16 (node index within block; -1 = pad)
        block_order: list of per-position original block ids
    """
    E = dst.shape[0]
    core = dst // NPC
    blk = (dst % NPC) // P
    nloc = (dst % NPC) % P

    order = np.argsort(dst, kind="stable")

    per_core_raw = []
    counts = np.zeros((NCORES, NBLK), np.int64)
    for c in range(NCORES):
        sel = order[(core[order] == c)]
        b = blk[sel]
        blists = []
        for j in range(NBLK):
            eb = sel[b == j]
            blists.append(eb)
            counts[c, j] = len(eb)
        per_core_raw.append(blists)

    tiles = (counts + P - 1) // P           # [NCORES, NBLK]
    tiles = np.maximum(tiles, 1)
    # sort each core's blocks by tile count desc; per-position count = max
    orders = [list(np.argsort(-tiles[c], kind="stable")) for c in range(NCORES)]
    T_sched = []
    for j in range(NBLK):
        t = max(tiles[c, orders[c][j]] for c in range(NCORES))
        t = int(t + (t & 1))                # round up to even
        T_sched.append(max(t, 2))

    chunks = []
    for t in T_sched:
        ch = [4] * (t // 4)
        if t % 4:
            ch.append(t % 4)                # t even => remainder 2
        chunks.append(ch)

    TT = sum(T_sched)
    per_core = []
    for c in range(NCORES):
        perm = np.full(TT * P, -1, np.int64)
        dloc = np.full(TT * P, -1, np.int16)
        off = 0
        for j in range(NBLK):
            bj = orders[c][j]
            eb = per_core_raw[c][bj]
            perm[off:off + len(eb)] = eb
            dloc[off:off + len(eb)] = nloc[eb].astype(np.int16)
            off += T_sched[j] * P
        per_core.append(dict(edge_perm=perm, dst_local=dloc, block_order=orders[c]))
    return T_sched, chunks, TT, per_core


def _host_prep(inputs):
    bond = np.asarray(inputs["bond_embedding"], np.float32)
    nemb = np.asarray(inputs["node_embedding"], np.float32)
    ba = np.asarray(inputs["basic_attn"], np.float32).reshape(-1)
    Wk = np.asarray(inputs["Wk"], np.float32)
    Wq = np.asarray(inputs["Wq"], np.float32)
    Wv = np.asarray(inputs["Wv"], np.float32)
    W_dis = np.asarray(inputs["W_dis"], np.float32)
    W_beta = np.asarray(inputs["W_beta"], np.float32).reshape(-1)
    W1 = np.asarray(inputs["W1"], np.float32)
    W2 = np.asarray(inputs["W2"], np.float32)
    src = np.asarray(inputs["src"], np.int64)
    dst = np.asarray(inputs["dst"], np.int64)

    T_sched, chunks, TT, per_core = _schedule(dst)

    # weights (replicated)
    scale = 1.0 / np.sqrt(np.float32(DH))
    consts = dict(
        WkT=np.ascontiguousarray(Wk.T).astype(BF16),
        WqT=np.ascontiguousarray((Wq * scale).T).astype(BF16),
        WvT=np.ascontiguousarray(Wv.T).astype(BF16),
        W1T=np.ascontiguousarray(W1.T).astype(BF16),        # [128, 256]
        W2T=np.ascontiguousarray(W2.T).astype(BF16),        # [256, 128]
        wdis=np.broadcast_to(W_dis.reshape(1, H), (P, H)).astype(np.float32).copy(),
        wbh=np.broadcast_to((W_beta[0:D] + W_beta[2 * D:3 * D]).reshape(1, D), (P, D)).astype(np.float32).copy(),
        wbx=np.broadcast_to((W_beta[D:2 * D] - W_beta[2 * D:3 * D]).reshape(1, D), (P, D)).astype(np.float32).copy(),
        ident=np.eye(P, dtype=np.float32),
        ones4=np.ones((P, 4), BF16),
    )

    nembT = np.ascontiguousarray(nemb.T)    # [128, N]

    in_maps = []
    unperm = []
    for c in range(NCORES):
        pc = per_core[c]
        perm = pc["edge_perm"]
        safe = np.where(perm >= 0, perm, 0)

        # streams in [feat, edge] layout, bf16
        embsT = np.ascontiguousarray(nembT[:, src[safe]]).astype(BF16)
        embdT = np.ascontiguousarray(nembT[:, dst[safe]]).astype(BF16)
        bondT = np.ascontiguousarray(bond[safe].T).astype(BF16)

        # per-tile-partition arrays [128, TT]: element (p, t) = edge t*128+p
        ba_pm = np.ascontiguousarray(
            np.where(perm >= 0, ba[safe], 0.0).astype(np.float32).reshape(TT, P).T)
        dloc = pc["dst_local"].astype(np.int32).reshape(TT, P).T  # [128, TT]
        # local_scatter indices: within chunk, column = tile_in_chunk*128 + dst_local
        sidx = np.full((P, TT), -1, np.int32)
        off = 0
        for j in range(NBLK):
            for ch in chunks[j]:
                for t in range(ch):
                    col = off + t
                    d_ = dloc[:, col]
                    sidx[:, col] = np.where(d_ >= 0, t * P + d_, -1)
                off += ch
        sidx = sidx.astype(np.int16)

        # node-side: local x in [128, NBLK*128] partition-major by block,
        # following block_order (position j holds original block order[j])
        nx = np.zeros((P, NBLK * P), np.float32)
        base = c * NPC
        for j, bj in enumerate(pc["block_order"]):
            lo = base + bj * P
            hi = min(lo + P, base + NPC)
            n = hi - lo
            nx[:n, j * P:j * P + P] = nemb[lo:hi, :]

        in_maps.append(dict(
            embsT=embsT, embdT=embdT, bondT=bondT,
            ba=ba_pm, sidx=sidx, nx=nx, **consts))
        unperm.append(pc["block_order"])

    return T_sched, chunks, TT, in_maps, unperm


# --------------------------------------------------------------------------
# device program
# --------------------------------------------------------------------------

def build_program(T_sched, chunks, TT):
    nc = bacc.Bacc("TRN2", target_bir_lowering=False, debug=False,
                   num_devices=NCORES)
    f32 = mybir.dt.float32
    bf16 = mybir.dt.bfloat16
    i16 = mybir.dt.int16
    AL = mybir.AluOpType
    AF = mybir.ActivationFunctionType

    embsT_in = nc.dram_tensor("embsT", [P, TT * P], bf16, kind="ExternalInput")
    embdT_in = nc.dram_tensor("embdT", [P, TT * P], bf16, kind="ExternalInput")
    bondT_in = nc.dram_tensor("bondT", [P, TT * P], bf16, kind="ExternalInput")
    ba_in = nc.dram_tensor("ba", [P, TT], f32, kind="ExternalInput")
    sidx_in = nc.dram_tensor("sidx", [P, TT], i16, kind="ExternalInput")
    nx_in = nc.dram_tensor("nx", [P, NBLK * P], f32, kind="ExternalInput")
    WkT_in = nc.dram_tensor("WkT", [P, P], bf16, kind="ExternalInput")
    WqT_in = nc.dram_tensor("WqT", [P, P], bf16, kind="ExternalInput")
    WvT_in = nc.dram_tensor("WvT", [P, P], bf16, kind="ExternalInput")
    W1T_in = nc.dram_tensor("W1T", [P, D_FF], bf16, kind="ExternalInput")
    W2T_in = nc.dram_tensor("W2T", [D_FF, P], bf16, kind="ExternalInput")
    wdis_in = nc.dram_tensor("wdis", [P, H], f32, kind="ExternalInput")
    wbh_in = nc.dram_tensor("wbh", [P, D], f32, kind="ExternalInput")
    wbx_in = nc.dram_tensor("wbx", [P, D], f32, kind="ExternalInput")
    ident_in = nc.dram_tensor("ident", [P, P], f32, kind="ExternalInput")
    ones4_in = nc.dram_tensor("ones4", [P, 4], bf16, kind="ExternalInput")

    out_dram = nc.dram_tensor("out", [P, NBLK * P], f32, kind="ExternalOutput")

    with ExitStack() as ctx:
        tc = ctx.enter_context(tile.TileContext(nc))
        cst = ctx.enter_context(tc.tile_pool(name="cst", bufs=1))
        res = ctx.enter_context(tc.tile_pool(name="res", bufs=1))
        edg = ctx.enter_context(tc.tile_pool(name="edg", bufs=3))
        sml = ctx.enter_context(tc.tile_pool(name="sml", bufs=4))
        wrk = ctx.enter_context(tc.tile_pool(name="wrk", bufs=3))
        epi = ctx.enter_context(tc.tile_pool(name="epi", bufs=3))
        psMM = ctx.enter_context(tc.tile_pool(name="psMM", bufs=6, space="PSUM"))
        psACC = ctx.enter_context(tc.tile_pool(name="psACC", bufs=2, space="PSUM"))

        def load_const(inp, shape, dtype, tag):
            t = cst.tile(shape, dtype, tag=tag)
            nc.sync.dma_start(out=t[:], in_=inp[:, :])
            return t

        WkT = load_const(WkT_in, [P, P], bf16, "WkT")
        WqT = load_const(WqT_in, [P, P], bf16, "WqT")
        WvT = load_const(WvT_in, [P, P], bf16, "WvT")
        W1T = load_const(W1T_in, [P, D_FF], bf16, "W1T")
        wdis = load_const(wdis_in, [P, H], f32, "wdis")
        wbh = load_const(wbh_in, [P, D], f32, "wbh")
        wbx = load_const(wbx_in, [P, D], f32, "wbx")
        ident = load_const(ident_in, [P, P], f32, "ident")
        ones4 = load_const(ones4_in, [P, 4], bf16, "ones4")

        eps_t = cst.tile([P, 1], f32, tag="eps")
        nc.gpsimd.memset(eps_t[:], EPS)

        W2Ta = cst.tile([P, P], bf16, tag="W2Ta")
        nc.sync.dma_start(out=W2Ta[:], in_=W2T_in[0:P, :])
        W2Tb = cst.tile([P, P], bf16, tag="W2Tb")
        nc.sync.dma_start(out=W2Tb[:], in_=W2T_in[P:2 * P, :])

        nx_res = res.tile([P, NBLK * P], f32)
        nc.sync.dma_start(out=nx_res[:], in_=nx_in[:, :])

        off = 0
        for j in range(NBLK):
            T = T_sched[j]
            e0 = off * P

            bond_t = edg.tile([P, T * P], bf16, tag="bond")
            nc.sync.dma_start(out=bond_t[:], in_=bondT_in[:, e0:e0 + T * P])
            embs_t = edg.tile([P, T * P], bf16, tag="embs")
            nc.sync.dma_start(out=embs_t[:], in_=embsT_in[:, e0:e0 + T * P])
            embd_t = edg.tile([P, T * P], bf16, tag="embd")
            nc.sync.dma_start(out=embd_t[:], in_=embdT_in[:, e0:e0 + T * P])
            ba_t = sml.tile([P, T], f32, tag="ba")
            nc.sync.dma_start(out=ba_t[:], in_=ba_in[:, off:off + T])
            sidx_t = sml.tile([P, T], i16, tag="sidx")
            nc.sync.dma_start(out=sidx_t[:], in_=sidx_in[:, off:off + T])

            att_t = sml.tile([P, T * H], f32, tag="att")
            wvp_t = wrk.tile([P, T * W136], bf16, tag="wvp")

            # bias = ba*wdis (no dep on att; runs early on Pool)
            bias_t = sml.tile([P, T * H], f32, tag="bias")
            nc.gpsimd.tensor_tensor(
                out=bias_t[:].rearrange("p (t h) -> p t h", h=H),
                in0=ba_t[:].unsqueeze(-1).to_broadcast([P, T, H]),
                in1=wdis[:].unsqueeze(1).to_broadcast([P, T, H]),
                op=AL.mult)

            # pass 1: attention logits
            c0 = 0
            for cw in chunks[j]:
                q_ps = psMM.tile([P, cw * P], f32, tag="mm")
                for t in range(cw):
                    nc.tensor.matmul(
                        out=q_ps[:, t * P:(t + 1) * P],
                        lhsT=embd_t[:, (c0 + t) * P:(c0 + t + 1) * P],
                        rhs=WqT[:], start=True, stop=True)
                q_sb = wrk.tile([P, cw * P], bf16, tag="qsb")
                nc.scalar.copy(out=q_sb[:], in_=q_ps[:])
                k_ps = psMM.tile([P, cw * P], f32, tag="mm")
                for t in range(cw):
                    nc.tensor.matmul(
                        out=k_ps[:, t * P:(t + 1) * P],
                        lhsT=embs_t[:, (c0 + t) * P:(c0 + t + 1) * P],
                        rhs=WkT[:], start=True, stop=True)
                kq_t = wrk.tile([P, cw * P], bf16, tag="kq")
                nc.vector.tensor_tensor(out=kq_t[:], in0=k_ps[:], in1=q_sb[:],
                                        op=AL.mult)
                nc.vector.tensor_reduce(
                    out=att_t[:].rearrange("p (t h) -> p t h", h=H)[:, c0:c0 + cw, :],
                    in_=kq_t[:].rearrange("p (t h d) -> p t h d", h=H, d=DH),
                    axis=mybir.AxisListType.X, op=AL.add)
                c0 += cw

            # logits -> p, written into the p-slots of wvp
            nc.vector.tensor_tensor(out=bias_t[:], in0=bias_t[:], in1=att_t[:],
                                    op=AL.add)
            p_t = sml.tile([P, T * H], bf16, tag="pexp")
            nc.scalar.activation(out=p_t[:], in_=bias_t[:], func=AF.Exp)
            nc.vector.tensor_copy(
                out=wvp_t[:].rearrange("p (t w) -> p t w", w=W136)[:, :, D:W136],
                in_=p_t[:].rearrange("p (t h) -> p t h", h=H))

            # pass 2: v, wv, one-hot scatter
            acc_ps = psACC.tile([P, 512], f32, tag="acc")
            c0 = 0
            for cw in chunks[j]:
                v_ps = psMM.tile([P, cw * P], f32, tag="mm")
                for t in range(cw):
                    nc.tensor.matmul(
                        out=v_ps[:, t * P:(t + 1) * P],
                        lhsT=bond_t[:, (c0 + t) * P:(c0 + t + 1) * P],
                        rhs=WvT[:], start=True, stop=True)
                s_t = wrk.tile([P, cw * P], bf16, tag="sh")
                nc.gpsimd.local_scatter(
                    out_ap=s_t[:], data_ap=ones4[:, 0:cw],
                    idxs_ap=sidx_t[:, c0:c0 + cw],
                    channels=P, num_elems=cw * P, num_idxs=cw)
                nc.vector.tensor_tensor(
                    out=wvp_t[:].rearrange("p (t w) -> p t w", w=W136)
                        [:, c0:c0 + cw, 0:D].rearrange("p t (h d) -> p t h d", h=H),
                    in0=v_ps[:].rearrange("p (t h d) -> p t h d", h=H, d=DH),
                    in1=wvp_t[:].rearrange("p (t w) -> p t w", w=W136)
                        [:, c0:c0 + cw, D:W136].unsqueeze(-1).to_broadcast([P, cw, H, DH]),
                    op=AL.mult)
                for t in range(cw):
                    gt = c0 + t
                    nc.tensor.matmul(
                        out=acc_ps[:, 0:W136],
                        lhsT=s_t[:, t * P:(t + 1) * P],
                        rhs=wvp_t[:, gt * W136:(gt + 1) * W136],
                        start=(gt == 0), stop=(gt == T - 1))
                c0 += cw

            # ---- per-block node epilogue (overlaps with later blocks) ----
            nxs = nx_res[:, j * P:(j + 1) * P]
            acc_t = epi.tile([P, W136], f32, tag="acct")
            nc.scalar.copy(out=acc_t[:], in_=acc_ps[:, 0:W136])

            rs_t = sml.tile([P, H], f32, tag="rs")
            nc.gpsimd.tensor_scalar(out=rs_t[:], in0=acc_t[:, D:W136],
                                    scalar1=SEPS, scalar2=None, op0=AL.add)
            nc.vector.reciprocal(out=rs_t[:], in_=rs_t[:])
            he_t = epi.tile([P, P], f32, tag="he")
            nc.gpsimd.tensor_tensor(
                out=he_t[:].rearrange("p (h d) -> p h d", h=H),
                in0=acc_t[:, 0:D].rearrange("p (h d) -> p h d", h=H),
                in1=rs_t[:].unsqueeze(-1).to_broadcast([P, H, DH]),
                op=AL.mult)

            # beta = sigmoid(he.wbh + x.wbx)
            z_t = sml.tile([P, 4], f32, tag="z")
            scr1 = epi.tile([P, P], f32, tag="scr1")
            nc.gpsimd.scalar_tensor_tensor(
                out=scr1[:], in0=he_t[:], scalar=1.0, in1=wbh[:],
                op0=AL.mult, op1=AL.mult, accum_out=z_t[:, 0:1])
            scr2 = epi.tile([P, P], f32, tag="scr2")
            nc.gpsimd.scalar_tensor_tensor(
                out=scr2[:], in0=nxs, scalar=1.0, in1=wbx[:],
                op0=AL.mult, op1=AL.mult, accum_out=z_t[:, 1:2])
            nc.gpsimd.tensor_tensor(out=z_t[:, 2:3], in0=z_t[:, 0:1],
                                    in1=z_t[:, 1:2], op=AL.add)
            beta_t = sml.tile([P, 1], f32, tag="beta")
            nc.scalar.activation(out=beta_t[:], in_=z_t[:, 2:3], func=AF.Sigmoid)

            # he2 = he + beta*(x - he)
            d_t = epi.tile([P, P], f32, tag="d")
            nc.gpsimd.tensor_tensor(out=d_t[:], in0=nxs, in1=he_t[:],
                                    op=AL.subtract)
            he2_t = epi.tile([P, P], f32, tag="he2")
            nc.gpsimd.scalar_tensor_tensor(
                out=he2_t[:], in0=d_t[:], scalar=beta_t[:, 0:1], in1=he_t[:],
                op0=AL.mult, op1=AL.add)

            def layer_norm(src_t, dst_tag, center_eng):
                """dst = LN(src) for one block; returns dst tile."""
                negmu = sml.tile([P, 1], f32, tag="negmu")
                nc.vector.tensor_reduce(out=negmu[:], in_=src_t[:],
                                        axis=mybir.AxisListType.X, op=AL.add,
                                        negate=True)
                nc.vector.tensor_scalar(out=negmu[:], in0=negmu[:],
                                        scalar1=1.0 / P, scalar2=None,
                                        op0=AL.mult)
                hc_t = epi.tile([P, P], f32, tag=dst_tag + "hc")
                nc.gpsimd.tensor_tensor(out=hc_t[:], in0=src_t[:],
                                        in1=negmu[:, 0:1].to_broadcast([P, P]),
                                        op=AL.add)
                sq_t = epi.tile([P, P], f32, tag=dst_tag + "sq")
                var_t = sml.tile([P, 1], f32, tag="var")
                nc.gpsimd.scalar_tensor_tensor(
                    out=sq_t[:], in0=hc_t[:], scalar=1.0, in1=hc_t[:],
                    op0=AL.mult, op1=AL.mult, accum_out=var_t[:])
                nc.scalar.activation(out=var_t[:], in_=var_t[:], func=AF.Sqrt,
                                     bias=eps_t[:, 0:1], scale=1.0 / P)
                nc.vector.reciprocal(out=var_t[:], in_=var_t[:])
                y_t = epi.tile([P, P], f32, tag=dst_tag)
                nc.vector.tensor_scalar(out=y_t[:], in0=hc_t[:],
                                        scalar1=var_t[:, 0:1], scalar2=None,
                                        op0=AL.mult)
                return y_t

            y_t = layer_norm(he2_t, "y", None)

            yT_ps = psMM.tile([P, P], f32, tag="mm")
            nc.tensor.transpose(out=yT_ps[:], in_=y_t[:], identity=ident[:])
            yT_sb = epi.tile([P, P], bf16, tag="yts")
            nc.scalar.copy(out=yT_sb[:], in_=yT_ps[:])
            h1a_ps = psMM.tile([P, P], f32, tag="mm")
            nc.tensor.matmul(out=h1a_ps[:], lhsT=W1T[:, 0:P], rhs=yT_sb[:],
                             start=True, stop=True)
            h1b_ps = psMM.tile([P, P], f32, tag="mm")
            nc.tensor.matmul(out=h1b_ps[:], lhsT=W1T[:, P:2 * P], rhs=yT_sb[:],
                             start=True, stop=True)
            h1a_sb = epi.tile([P, P], bf16, tag="h1as")
            nc.scalar.activation(out=h1a_sb[:], in_=h1a_ps[:], func=AF.Relu)
            h1b_sb = epi.tile([P, P], bf16, tag="h1bs")
            nc.scalar.activation(out=h1b_sb[:], in_=h1b_ps[:], func=AF.Relu)
            h2_ps = psMM.tile([P, P], f32, tag="mm")
            nc.tensor.matmul(out=h2_ps[:], lhsT=h1a_sb[:], rhs=W2Ta[:],
                             start=True, stop=False)
            nc.tensor.matmul(out=h2_ps[:], lhsT=h1b_sb[:], rhs=W2Tb[:],
                             start=False, stop=True)
            o_t = epi.tile([P, P], f32, tag="o")
            nc.vector.tensor_tensor(out=o_t[:], in0=h2_ps[:], in1=he2_t[:],
                                    op=AL.add)

            out_t = layer_norm(o_t, "out", None)
            nc.sync.dma_start(out=out_dram[:, j * P:(j + 1) * P], in_=out_t[:])

            off += T

    nc.finalize()
    return nc


# --------------------------------------------------------------------------
# entry point
# --------------------------------------------------------------------------

def kernel(**inputs):
    import os
    T_sched, chunks, TT, in_maps, block_orders = _host_prep(inputs)

    key = tuple(T_sched)
    if key not in _CACHE:
        _CACHE[key] = build_program(T_sched, chunks, TT)
    nc = _CACHE[key]

    trace = bool(os.environ.get("BASS_KERNEL_TRACE"))
    tmpdir = os.environ.get("BASS_KERNEL_TRACE_DIR") or None
    results = run_bass_kernel_spmd(nc, in_maps, core_ids=list(range(NCORES)),
                                   trace=trace, tmpdir=tmpdir)
    if trace and results.exec_time_ns is not None:
        print(f"HW exec time: {results.exec_time_ns} ns")

    out = np.zeros((N_NODES, D), np.float32)
    for c in range(NCORES):
        o = results.results[c]["out"]          # [128, NBLK*128]
        base = c * NPC
        for j, bj in enumerate(block_orders[c]):
            lo = base + bj * P
            hi = min(lo + P, base + NPC)
            n = hi - lo
            out[lo:hi, :] = o[:n, j * P:j * P + P]
    return out


# revision 16
# speedup vs baseline: 1.2670x; 1.2670x over previous
"""Trainium2 Bass kernel for nn_Bond2AtomLayer (GNN message passing).

Strategy (8-core SPMD, dst-node partitioned):
- Host: sort edges by dst, partition nodes into 8 ranges of 6250; each core
  owns the edges whose dst falls in its range. Within a core, edges are
  grouped into 128-node "blocks" (49 per core), each block's edges padded to
  a whole number of 128-edge tiles. Blocks are assigned to fixed program
  positions with a global per-position tile count (max over cores) so all
  8 cores run one identical program.
- Host pre-gathers node_emb[src], node_emb[dst] and bond rows into per-core
  edge-order streams, transposed to [feat, edge] (bf16) so the device needs
  no gather at all — k/q/v are computed per 128-edge tile by PE matmuls
  against the small weight matrices.
- Edge softmax: att[e,h] = sum_d k[e,hd]*q[e,hd] (DVE mult + grouped reduce),
  logits = att/4 + ba*W_dis (1/4 folded into Wq), p = exp(logits) without
  max-subtraction (logits are O(1); softmax is shift-invariant so this
  matches the reference numerically).
- Scatter-sum: per tile a one-hot S[e,n] (bf16, built by GPSIMD local_scatter)
  and one PE matmul accumulates [wv | p] into PSUM per 128-node block;
  ft = wsum / s, then beta-gating, LN, FFN, LN on-chip (node-major, batched).
"""
import sys

sys.path.insert(0, "/opt/trn_rl_repo")

import numpy as np
import ml_dtypes
from contextlib import ExitStack

import concourse.bass as bass
import concourse.tile as tile
from concourse import bacc, mybir
from concourse.bass_utils import run_bass_kernel_spmd

BF16 = ml_dtypes.bfloat16

N_NODES = 50000
N_EDGES = 800000
D = 128
H = 8
DH = 16
D_FF = 256
P = 128
NCORES = 8
NPC = N_NODES // NCORES        # 6250 nodes per core
NBLK = (NPC + P - 1) // P      # 49 blocks per core (last has 106 nodes)
NPAD = NBLK * P                # 6272
EPS = 1e-5
SEPS = 1e-30                   # guard for 1/s on isolated nodes
W136 = D + H                   # 136: [wv | p] scatter payload width

_CACHE = {}


# --------------------------------------------------------------------------
# host-side scheduling
# --------------------------------------------------------------------------

def _schedule(dst):
    """Partition edges by dst; build per-core block schedules.

    Returns:
      T_sched: list of per-position tile counts (same for all cores)
      chunks:  list of chunk widths per position (even, <=4 each)
      per_core: list of dicts with keys:
        edge_perm [TT*128] int64 (index into full edge list; -1 = pad)
        dst_local [TT*128] int16 (node index within block; -1 = pad)
        block_order: list of per-position original block ids
    """
    E = dst.shape[0]
    core = dst // NPC
    blk = (dst % NPC) // P
    nloc = (dst % NPC) % P

    order = np.argsort(dst, kind="stable")

    per_core_raw = []
    counts = np.zeros((NCORES, NBLK), np.int64)
    for c in range(NCORES):
        sel = order[(core[order] == c)]
        b = blk[sel]
        blists = []
        for j in range(NBLK):
            eb = sel[b == j]
            blists.append(eb)
            counts[c, j] = len(eb)
        per_core_raw.append(blists)

    tiles = (counts + P - 1) // P           # [NCORES, NBLK]
    tiles = np.maximum(tiles, 1)
    # sort each core's blocks by tile count desc; per-position count = max
    orders = [list(np.argsort(-tiles[c], kind="stable")) for c in range(NCORES)]
    T_sched = []
    for j in range(NBLK):
        t = max(tiles[c, orders[c][j]] for c in range(NCORES))
        t = int(t + (t & 1))                # round up to even
        T_sched.append(max(t, 2))

    chunks = []
    for t in T_sched:
        ch = [4] * (t // 4)
        if t % 4:
            ch.append(t % 4)                # t even => remainder 2
        chunks.append(ch)

    TT = sum(T_sched)
    per_core = []
    for c in range(NCORES):
        perm = np.full(TT * P, -1, np.int64)
        dloc = np.full(TT * P, -1, np.int16)
        off = 0
        for j in range(NBLK):
            bj = orders[c][j]
            eb = per_core_raw[c][bj]
            perm[off:off + len(eb)] = eb
            dloc[off:off + len(eb)] = nloc[eb].astype(np.int16)
            off += T_sched[j] * P
        per_core.append(dict(edge_perm=perm, dst_local=dloc, block_order=orders[c]))
    return T_sched, chunks, TT, per_core


def _host_prep(inputs):
    bond = np.asarray(inputs["bond_embedding"], np.float32)
    nemb = np.asarray(inputs["node_embedding"], np.float32)
    ba = np.asarray(inputs["basic_attn"], np.float32).reshape(-1)
    Wk = np.asarray(inputs["Wk"], np.float32)
    Wq = np.asarray(inputs["Wq"], np.float32)
    Wv = np.asarray(inputs["Wv"], np.float32)
    W_dis = np.asarray(inputs["W_dis"], np.float32)
    W_beta = np.asarray(inputs["W_beta"], np.float32).reshape(-1)
    W1 = np.asarray(inputs["W1"], np.float32)
    W2 = np.asarray(inputs["W2"], np.float32)
    src = np.asarray(inputs["src"], np.int64)
    dst = np.asarray(inputs["dst"], np.int64)

    T_sched, chunks, TT, per_core = _schedule(dst)

    # weights (replicated)
    scale = 1.0 / np.sqrt(np.float32(DH))
    consts = dict(
        WkT=np.ascontiguousarray(Wk.T).astype(BF16),
        WqT=np.ascontiguousarray((Wq * scale).T).astype(BF16),
        WvT=np.ascontiguousarray(Wv.T).astype(BF16),
        W1T=np.ascontiguousarray(W1.T).astype(BF16),        # [128, 256]
        W2T=np.ascontiguousarray(W2.T).astype(BF16),        # [256, 128]
        wdis=np.broadcast_to(W_dis.reshape(1, H), (P, H)).astype(np.float32).copy(),
        wbh=np.broadcast_to((W_beta[0:D] + W_beta[2 * D:3 * D]).reshape(1, D), (P, D)).astype(np.float32).copy(),
        wbx=np.broadcast_to((W_beta[D:2 * D] - W_beta[2 * D:3 * D]).reshape(1, D), (P, D)).astype(np.float32).copy(),
        ident=np.eye(P, dtype=np.float32),
        ones4=np.ones((P, 4), BF16),
    )

    nembT = np.ascontiguousarray(nemb.T)    # [128, N]

    in_maps = []
    unperm = []
    for c in range(NCORES):
        pc = per_core[c]
        perm = pc["edge_perm"]
        safe = np.where(perm >= 0, perm, 0)

        # streams in [feat, edge] layout, bf16
        embsT = np.ascontiguousarray(nembT[:, src[safe]]).astype(BF16)
        embdT = np.ascontiguousarray(nembT[:, dst[safe]]).astype(BF16)
        bondT = np.ascontiguousarray(bond[safe].T).astype(BF16)

        # per-tile-partition arrays [128, TT]: element (p, t) = edge t*128+p
        ba_pm = np.ascontiguousarray(
            np.where(perm >= 0, ba[safe], 0.0).astype(np.float32).reshape(TT, P).T)
        dloc = pc["dst_local"].astype(np.int32).reshape(TT, P).T  # [128, TT]
        # local_scatter indices: within chunk, column = tile_in_chunk*128 + dst_local
        sidx = np.full((P, TT), -1, np.int32)
        off = 0
        for j in range(NBLK):
            for ch in chunks[j]:
                for t in range(ch):
                    col = off + t
                    d_ = dloc[:, col]
                    sidx[:, col] = np.where(d_ >= 0, t * P + d_, -1)
                off += ch
        sidx = sidx.astype(np.int16)

        # node-side: local x in [128, NBLK*128] partition-major by block,
        # following block_order (position j holds original block order[j])
        nx = np.zeros((P, NBLK * P), np.float32)
        base = c * NPC
        for j, bj in enumerate(pc["block_order"]):
            lo = base + bj * P
            hi = min(lo + P, base + NPC)
            n = hi - lo
            nx[:n, j * P:j * P + P] = nemb[lo:hi, :]

        in_maps.append(dict(
            embsT=embsT, embdT=embdT, bondT=bondT,
            ba=ba_pm, sidx=sidx, nx=nx, **consts))
        unperm.append(pc["block_order"])

    return T_sched, chunks, TT, in_maps, unperm


# --------------------------------------------------------------------------
# device program
# --------------------------------------------------------------------------

def build_program(T_sched, chunks, TT):
    nc = bacc.Bacc("TRN2", target_bir_lowering=False, debug=False,
                   num_devices=NCORES)
    f32 = mybir.dt.float32
    bf16 = mybir.dt.bfloat16
    i16 = mybir.dt.int16
    AL = mybir.AluOpType
    AF = mybir.ActivationFunctionType

    embsT_in = nc.dram_tensor("embsT", [P, TT * P], bf16, kind="ExternalInput")
    embdT_in = nc.dram_tensor("embdT", [P, TT * P], bf16, kind="ExternalInput")
    bondT_in = nc.dram_tensor("bondT", [P, TT * P], bf16, kind="ExternalInput")
    ba_in = nc.dram_tensor("ba", [P, TT], f32, kind="ExternalInput")
    sidx_in = nc.dram_tensor("sidx", [P, TT], i16, kind="ExternalInput")
    nx_in = nc.dram_tensor("nx", [P, NBLK * P], f32, kind="ExternalInput")
    WkT_in = nc.dram_tensor("WkT", [P, P], bf16, kind="ExternalInput")
    WqT_in = nc.dram_tensor("WqT", [P, P], bf16, kind="ExternalInput")
    WvT_in = nc.dram_tensor("WvT", [P, P], bf16, kind="ExternalInput")
    W1T_in = nc.dram_tensor("W1T", [P, D_FF], bf16, kind="ExternalInput")
    W2T_in = nc.dram_tensor("W2T", [D_FF, P], bf16, kind="ExternalInput")
    wdis_in = nc.dram_tensor("wdis", [P, H], f32, kind="ExternalInput")
    wbh_in = nc.dram_tensor("wbh", [P, D], f32, kind="ExternalInput")
    wbx_in = nc.dram_tensor("wbx", [P, D], f32, kind="ExternalInput")
    ident_in = nc.dram_tensor("ident", [P, P], f32, kind="ExternalInput")
    ones4_in = nc.dram_tensor("ones4", [P, 4], bf16, kind="ExternalInput")

    out_dram = nc.dram_tensor("out", [P, NBLK * P], f32, kind="ExternalOutput")

    with ExitStack() as ctx:
        tc = ctx.enter_context(tile.TileContext(nc))
        cst = ctx.enter_context(tc.tile_pool(name="cst", bufs=1))
        res = ctx.enter_context(tc.tile_pool(name="res", bufs=1))
        edg = ctx.enter_context(tc.tile_pool(name="edg", bufs=3))
        sml = ctx.enter_context(tc.tile_pool(name="sml", bufs=4))
        wrk = ctx.enter_context(tc.tile_pool(name="wrk", bufs=3))
        epi = ctx.enter_context(tc.tile_pool(name="epi", bufs=3))
        psMM = ctx.enter_context(tc.tile_pool(name="psMM", bufs=4, space="PSUM"))
        psFFN = ctx.enter_context(tc.tile_pool(name="psFFN", bufs=2, space="PSUM"))
        psACC = ctx.enter_context(tc.tile_pool(name="psACC", bufs=2, space="PSUM"))

        def load_const(inp, shape, dtype, tag):
            t = cst.tile(shape, dtype, tag=tag)
            nc.sync.dma_start(out=t[:], in_=inp[:, :])
            return t

        WkT = load_const(WkT_in, [P, P], bf16, "WkT")
        WqT = load_const(WqT_in, [P, P], bf16, "WqT")
        WvT = load_const(WvT_in, [P, P], bf16, "WvT")
        W1T = load_const(W1T_in, [P, D_FF], bf16, "W1T")
        wdis = load_const(wdis_in, [P, H], f32, "wdis")
        wbh = load_const(wbh_in, [P, D], f32, "wbh")
        wbx = load_const(wbx_in, [P, D], f32, "wbx")
        ident = load_const(ident_in, [P, P], f32, "ident")
        ones4 = load_const(ones4_in, [P, 4], bf16, "ones4")

        eps_t = cst.tile([P, 1], f32, tag="eps")
        nc.gpsimd.memset(eps_t[:], EPS)

        W2Ta = cst.tile([P, P], bf16, tag="W2Ta")
        nc.sync.dma_start(out=W2Ta[:], in_=W2T_in[0:P, :])
        W2Tb = cst.tile([P, P], bf16, tag="W2Tb")
        nc.sync.dma_start(out=W2Tb[:], in_=W2T_in[P:2 * P, :])

        nx_res = res.tile([P, NBLK * P], f32)
        nc.sync.dma_start(out=nx_res[:], in_=nx_in[:, :])

        off = 0
        for j in range(NBLK):
            T = T_sched[j]
            e0 = off * P

            bond_t = edg.tile([P, T * P], bf16, tag="bond")
            nc.sync.dma_start(out=bond_t[:], in_=bondT_in[:, e0:e0 + T * P])
            embs_t = edg.tile([P, T * P], bf16, tag="embs")
            nc.sync.dma_start(out=embs_t[:], in_=embsT_in[:, e0:e0 + T * P])
            embd_t = edg.tile([P, T * P], bf16, tag="embd")
            nc.sync.dma_start(out=embd_t[:], in_=embdT_in[:, e0:e0 + T * P])
            ba_t = sml.tile([P, T], f32, tag="ba")
            nc.sync.dma_start(out=ba_t[:], in_=ba_in[:, off:off + T])
            sidx_t = sml.tile([P, T], i16, tag="sidx")
            nc.sync.dma_start(out=sidx_t[:], in_=sidx_in[:, off:off + T])

            att_t = sml.tile([P, T * H], f32, tag="att")
            wvp_t = wrk.tile([P, T * W136], bf16, tag="wvp")

            # bias = ba*wdis (no dep on att; runs early on Pool)
            bias_t = sml.tile([P, T * H], f32, tag="bias")
            nc.gpsimd.tensor_tensor(
                out=bias_t[:].rearrange("p (t h) -> p t h", h=H),
                in0=ba_t[:].unsqueeze(-1).to_broadcast([P, T, H]),
                in1=wdis[:].unsqueeze(1).to_broadcast([P, T, H]),
                op=AL.mult)

            # pass 1: attention logits
            c0 = 0
            for cw in chunks[j]:
                q_ps = psMM.tile([P, cw * P], f32, tag="mm")
                for t in range(cw):
                    nc.tensor.matmul(
                        out=q_ps[:, t * P:(t + 1) * P],
                        lhsT=embd_t[:, (c0 + t) * P:(c0 + t + 1) * P],
                        rhs=WqT[:], start=True, stop=True)
                q_sb = wrk.tile([P, cw * P], bf16, tag="qsb")
                nc.scalar.copy(out=q_sb[:], in_=q_ps[:])
                k_ps = psMM.tile([P, cw * P], f32, tag="mm")
                for t in range(cw):
                    nc.tensor.matmul(
                        out=k_ps[:, t * P:(t + 1) * P],
                        lhsT=embs_t[:, (c0 + t) * P:(c0 + t + 1) * P],
                        rhs=WkT[:], start=True, stop=True)
                kq_t = wrk.tile([P, cw * P], bf16, tag="kq")
                nc.vector.tensor_tensor(out=kq_t[:], in0=k_ps[:], in1=q_sb[:],
                                        op=AL.mult)
                nc.vector.tensor_reduce(
                    out=att_t[:].rearrange("p (t h) -> p t h", h=H)[:, c0:c0 + cw, :],
                    in_=kq_t[:].rearrange("p (t h d) -> p t h d", h=H, d=DH),
                    axis=mybir.AxisListType.X, op=AL.add)
                c0 += cw

            # logits -> p, written into the p-slots of wvp
            nc.vector.tensor_tensor(out=bias_t[:], in0=bias_t[:], in1=att_t[:],
                                    op=AL.add)
            p_t = sml.tile([P, T * H], bf16, tag="pexp")
            nc.scalar.activation(out=p_t[:], in_=bias_t[:], func=AF.Exp)
            nc.vector.tensor_copy(
                out=wvp_t[:].rearrange("p (t w) -> p t w", w=W136)[:, :, D:W136],
                in_=p_t[:].rearrange("p (t h) -> p t h", h=H))

            # pass 2: v, wv, one-hot scatter
            acc_ps = psACC.tile([P, 512], f32, tag="acc")
            c0 = 0
            for cw in chunks[j]:
                v_ps = psMM.tile([P, cw * P], f32, tag="mm")
                for t in range(cw):
                    nc.tensor.matmul(
                        out=v_ps[:, t * P:(t + 1) * P],
                        lhsT=bond_t[:, (c0 + t) * P:(c0 + t + 1) * P],
                        rhs=WvT[:], start=True, stop=True)
                s_t = wrk.tile([P, cw * P], bf16, tag="sh")
                nc.gpsimd.local_scatter(
                    out_ap=s_t[:], data_ap=ones4[:, 0:cw],
                    idxs_ap=sidx_t[:, c0:c0 + cw],
                    channels=P, num_elems=cw * P, num_idxs=cw)
                nc.vector.tensor_tensor(
                    out=wvp_t[:].rearrange("p (t w) -> p t w", w=W136)
                        [:, c0:c0 + cw, 0:D].rearrange("p t (h d) -> p t h d", h=H),
                    in0=v_ps[:].rearrange("p (t h d) -> p t h d", h=H, d=DH),
                    in1=wvp_t[:].rearrange("p (t w) -> p t w", w=W136)
                        [:, c0:c0 + cw, D:W136].unsqueeze(-1).to_broadcast([P, cw, H, DH]),
                    op=AL.mult)
                for t in range(cw):
                    gt = c0 + t
                    nc.tensor.matmul(
                        out=acc_ps[:, 0:W136],
                        lhsT=s_t[:, t * P:(t + 1) * P],
                        rhs=wvp_t[:, gt * W136:(gt + 1) * W136],
                        start=(gt == 0), stop=(gt == T - 1))
                c0 += cw

            # ---- per-block node epilogue (overlaps with later blocks) ----
            nxs = nx_res[:, j * P:(j + 1) * P]
            acc_t = epi.tile([P, W136], f32, tag="acct")
            nc.scalar.copy(out=acc_t[:], in_=acc_ps[:, 0:W136])

            rs_t = sml.tile([P, H], f32, tag="rs")
            nc.gpsimd.tensor_scalar(out=rs_t[:], in0=acc_t[:, D:W136],
                                    scalar1=SEPS, scalar2=None, op0=AL.add)
            nc.vector.reciprocal(out=rs_t[:], in_=rs_t[:])
            he_t = epi.tile([P, P], f32, tag="he")
            nc.gpsimd.tensor_tensor(
                out=he_t[:].rearrange("p (h d) -> p h d", h=H),
                in0=acc_t[:, 0:D].rearrange("p (h d) -> p h d", h=H),
                in1=rs_t[:].unsqueeze(-1).to_broadcast([P, H, DH]),
                op=AL.mult)

            # beta = sigmoid(he.wbh + x.wbx)
            z_t = sml.tile([P, 4], f32, tag="z")
            scr1 = epi.tile([P, P], f32, tag="scr1")
            nc.gpsimd.scalar_tensor_tensor(
                out=scr1[:], in0=he_t[:], scalar=1.0, in1=wbh[:],
                op0=AL.mult, op1=AL.mult, accum_out=z_t[:, 0:1])
            scr2 = epi.tile([P, P], f32, tag="scr2")
            nc.gpsimd.scalar_tensor_tensor(
                out=scr2[:], in0=nxs, scalar=1.0, in1=wbx[:],
                op0=AL.mult, op1=AL.mult, accum_out=z_t[:, 1:2])
            nc.gpsimd.tensor_tensor(out=z_t[:, 2:3], in0=z_t[:, 0:1],
                                    in1=z_t[:, 1:2], op=AL.add)
            beta_t = sml.tile([P, 1], f32, tag="beta")
            nc.scalar.activation(out=beta_t[:], in_=z_t[:, 2:3], func=AF.Exp,
                                 scale=-1.0)
            nc.gpsimd.tensor_scalar(out=beta_t[:], in0=beta_t[:], scalar1=1.0,
                                    scalar2=None, op0=AL.add)
            nc.vector.reciprocal(out=beta_t[:], in_=beta_t[:])

            # he2 = he + beta*(x - he)
            d_t = epi.tile([P, P], f32, tag="d")
            nc.gpsimd.tensor_tensor(out=d_t[:], in0=nxs, in1=he_t[:],
                                    op=AL.subtract)
            he2_t = epi.tile([P, P], f32, tag="he2")
            nc.gpsimd.scalar_tensor_tensor(
                out=he2_t[:], in0=d_t[:], scalar=beta_t[:, 0:1], in1=he_t[:],
                op0=AL.mult, op1=AL.add)

            def layer_norm(src_t, dst_tag, center_eng):
                """dst = LN(src) for one block; returns dst tile."""
                negmu = sml.tile([P, 1], f32, tag="negmu")
                nc.vector.tensor_reduce(out=negmu[:], in_=src_t[:],
                                        axis=mybir.AxisListType.X, op=AL.add,
                                        negate=True)
                nc.vector.tensor_scalar(out=negmu[:], in0=negmu[:],
                                        scalar1=1.0 / P, scalar2=None,
                                        op0=AL.mult)
                hc_t = epi.tile([P, P], f32, tag=dst_tag + "hc")
                nc.gpsimd.tensor_tensor(out=hc_t[:], in0=src_t[:],
                                        in1=negmu[:, 0:1].to_broadcast([P, P]),
                                        op=AL.add)
                sq_t = epi.tile([P, P], f32, tag=dst_tag + "sq")
                var_t = sml.tile([P, 1], f32, tag="var")
                nc.gpsimd.scalar_tensor_tensor(
                    out=sq_t[:], in0=hc_t[:], scalar=1.0, in1=hc_t[:],
                    op0=AL.mult, op1=AL.mult, accum_out=var_t[:])
                nc.gpsimd.tensor_scalar(out=var_t[:], in0=var_t[:],
                                        scalar1=1.0 / P, scalar2=EPS,
                                        op0=AL.mult, op1=AL.add)
                nc.scalar.activation(out=var_t[:], in_=var_t[:], func=AF.Ln)
                nc.scalar.activation(out=var_t[:], in_=var_t[:], func=AF.Exp,
                                     scale=-0.5)
                y_t = epi.tile([P, P], f32, tag=dst_tag)
                nc.vector.tensor_scalar(out=y_t[:], in0=hc_t[:],
                                        scalar1=var_t[:, 0:1], scalar2=None,
                                        op0=AL.mult)
                return y_t

            y_t = layer_norm(he2_t, "y", None)

            yT_ps = psFFN.tile([P, P], f32, tag="ffn")
            nc.tensor.transpose(out=yT_ps[:], in_=y_t[:], identity=ident[:])
            yT_sb = epi.tile([P, P], bf16, tag="yts")
            nc.scalar.copy(out=yT_sb[:], in_=yT_ps[:])
            h1a_ps = psFFN.tile([P, P], f32, tag="ffn")
            nc.tensor.matmul(out=h1a_ps[:], lhsT=W1T[:, 0:P], rhs=yT_sb[:],
                             start=True, stop=True)
            h1b_ps = psFFN.tile([P, P], f32, tag="ffn")
            nc.tensor.matmul(out=h1b_ps[:], lhsT=W1T[:, P:2 * P], rhs=yT_sb[:],
                             start=True, stop=True)
            h1a_sb = epi.tile([P, P], bf16, tag="h1as")
            nc.scalar.activation(out=h1a_sb[:], in_=h1a_ps[:], func=AF.Relu)
            h1b_sb = epi.tile([P, P], bf16, tag="h1bs")
            nc.scalar.activation(out=h1b_sb[:], in_=h1b_ps[:], func=AF.Relu)
            h2_ps = psFFN.tile([P, P], f32, tag="ffn")
            nc.tensor.matmul(out=h2_ps[:], lhsT=h1a_sb[:], rhs=W2Ta[:],
                             start=True, stop=False)
            nc.tensor.matmul(out=h2_ps[:], lhsT=h1b_sb[:], rhs=W2Tb[:],
                             start=False, stop=True)
            o_t = epi.tile([P, P], f32, tag="o")
            nc.vector.tensor_tensor(out=o_t[:], in0=h2_ps[:], in1=he2_t[:],
                                    op=AL.add)

            out_t = layer_norm(o_t, "out", None)
            nc.sync.dma_start(out=out_dram[:, j * P:(j + 1) * P], in_=out_t[:])

            off += T

    nc.finalize()
    return nc


# --------------------------------------------------------------------------
# entry point
# --------------------------------------------------------------------------

def kernel(**inputs):
    import os
    T_sched, chunks, TT, in_maps, block_orders = _host_prep(inputs)

    key = tuple(T_sched)
    if key not in _CACHE:
        _CACHE[key] = build_program(T_sched, chunks, TT)
    nc = _CACHE[key]

    trace = bool(os.environ.get("BASS_KERNEL_TRACE"))
    tmpdir = os.environ.get("BASS_KERNEL_TRACE_DIR") or None
    results = run_bass_kernel_spmd(nc, in_maps, core_ids=list(range(NCORES)),
                                   trace=trace, tmpdir=tmpdir)
    if trace and results.exec_time_ns is not None:
        print(f"HW exec time: {results.exec_time_ns} ns")

    out = np.zeros((N_NODES, D), np.float32)
    for c in range(NCORES):
        o = results.results[c]["out"]          # [128, NBLK*128]
        base = c * NPC
        for j, bj in enumerate(block_orders[c]):
            lo = base + bj * P
            hi = min(lo + P, base + NPC)
            n = hi - lo
            out[lo:hi, :] = o[:n, j * P:j * P + P]
    return out


# revision 17
# speedup vs baseline: 1.4897x; 1.1758x over previous
"""Trainium2 Bass kernel for nn_Bond2AtomLayer (GNN message passing).

Strategy (8-core SPMD, dst-node partitioned):
- Host: sort edges by dst, partition nodes into 8 ranges of 6250; each core
  owns the edges whose dst falls in its range. Within a core, edges are
  grouped into 128-node "blocks" (49 per core), each block's edges padded to
  a whole number of 128-edge tiles. Blocks are assigned to fixed program
  positions with a global per-position tile count (max over cores) so all
  8 cores run one identical program.
- Host pre-gathers node_emb[src], node_emb[dst] and bond rows into per-core
  edge-order streams, transposed to [feat, edge] (bf16) so the device needs
  no gather at all — k/q/v are computed per 128-edge tile by PE matmuls
  against the small weight matrices.
- Edge softmax: att[e,h] = sum_d k[e,hd]*q[e,hd] (DVE mult + grouped reduce),
  logits = att/4 + ba*W_dis (1/4 folded into Wq), p = exp(logits) without
  max-subtraction (logits are O(1); softmax is shift-invariant so this
  matches the reference numerically).
- Scatter-sum: per tile a one-hot S[e,n] (bf16, built by GPSIMD local_scatter)
  and one PE matmul accumulates [wv | p] into PSUM per 128-node block;
  ft = wsum / s, then beta-gating, LN, FFN, LN on-chip (node-major, batched).
"""
import sys

sys.path.insert(0, "/opt/trn_rl_repo")

import numpy as np
import ml_dtypes
from contextlib import ExitStack

import concourse.bass as bass
import concourse.tile as tile
from concourse import bacc, mybir
from concourse.bass_utils import run_bass_kernel_spmd

BF16 = ml_dtypes.bfloat16

N_NODES = 50000
N_EDGES = 800000
D = 128
H = 8
DH = 16
D_FF = 256
P = 128
NCORES = 8
NPC = N_NODES // NCORES        # 6250 nodes per core
NBLK = (NPC + P - 1) // P      # 49 blocks per core (last has 106 nodes)
NPAD = NBLK * P                # 6272
EPS = 1e-5
SEPS = 1e-30                   # guard for 1/s on isolated nodes
W136 = D + H                   # 136: [wv | p] scatter payload width

_CACHE = {}


# --------------------------------------------------------------------------
# host-side scheduling
# --------------------------------------------------------------------------

def _schedule(dst):
    """Partition edges by dst; build per-core block schedules.

    Returns:
      T_sched: list of per-position tile counts (same for all cores)
      chunks:  list of chunk widths per position (even, <=4 each)
      per_core: list of dicts with keys:
        edge_perm [TT*128] int64 (index into full edge list; -1 = pad)
        dst_local [TT*128] int16 (node index within block; -1 = pad)
        block_order: list of per-position original block ids
    """
    E = dst.shape[0]
    core = dst // NPC
    blk = (dst % NPC) // P
    nloc = (dst % NPC) % P

    order = np.argsort(dst, kind="stable")

    per_core_raw = []
    counts = np.zeros((NCORES, NBLK), np.int64)
    for c in range(NCORES):
        sel = order[(core[order] == c)]
        b = blk[sel]
        blists = []
        for j in range(NBLK):
            eb = sel[b == j]
            blists.append(eb)
            counts[c, j] = len(eb)
        per_core_raw.append(blists)

    tiles = (counts + P - 1) // P           # [NCORES, NBLK]
    tiles = np.maximum(tiles, 1)
    # sort each core's blocks by tile count desc; per-position count = max
    orders = [list(np.argsort(-tiles[c], kind="stable")) for c in range(NCORES)]
    T_sched = []
    for j in range(NBLK):
        t = max(tiles[c, orders[c][j]] for c in range(NCORES))
        t = int(t + (t & 1))                # round up to even
        T_sched.append(max(t, 2))

    chunks = []
    for t in T_sched:
        ch = [4] * (t // 4)
        if t % 4:
            ch.append(t % 4)                # t even => remainder 2
        chunks.append(ch)

    TT = sum(T_sched)
    per_core = []
    for c in range(NCORES):
        perm = np.full(TT * P, -1, np.int64)
        dloc = np.full(TT * P, -1, np.int16)
        off = 0
        for j in range(NBLK):
            bj = orders[c][j]
            eb = per_core_raw[c][bj]
            perm[off:off + len(eb)] = eb
            dloc[off:off + len(eb)] = nloc[eb].astype(np.int16)
            off += T_sched[j] * P
        per_core.append(dict(edge_perm=perm, dst_local=dloc, block_order=orders[c]))
    return T_sched, chunks, TT, per_core


def _host_prep(inputs):
    bond = np.asarray(inputs["bond_embedding"], np.float32)
    nemb = np.asarray(inputs["node_embedding"], np.float32)
    ba = np.asarray(inputs["basic_attn"], np.float32).reshape(-1)
    Wk = np.asarray(inputs["Wk"], np.float32)
    Wq = np.asarray(inputs["Wq"], np.float32)
    Wv = np.asarray(inputs["Wv"], np.float32)
    W_dis = np.asarray(inputs["W_dis"], np.float32)
    W_beta = np.asarray(inputs["W_beta"], np.float32).reshape(-1)
    W1 = np.asarray(inputs["W1"], np.float32)
    W2 = np.asarray(inputs["W2"], np.float32)
    src = np.asarray(inputs["src"], np.int64)
    dst = np.asarray(inputs["dst"], np.int64)

    T_sched, chunks, TT, per_core = _schedule(dst)

    # weights (replicated)
    scale = 1.0 / np.sqrt(np.float32(DH))
    consts = dict(
        WkT=np.ascontiguousarray(Wk.T).astype(BF16),
        WqT=np.ascontiguousarray((Wq * scale).T).astype(BF16),
        WvT=np.ascontiguousarray(Wv.T).astype(BF16),
        W1T=np.ascontiguousarray(W1.T).astype(BF16),        # [128, 256]
        W2T=np.ascontiguousarray(W2.T).astype(BF16),        # [256, 128]
        wdis=np.broadcast_to(W_dis.reshape(1, H), (P, H)).astype(np.float32).copy(),
        wbh=np.broadcast_to((W_beta[0:D] + W_beta[2 * D:3 * D]).reshape(1, D), (P, D)).astype(np.float32).copy(),
        wbx=np.broadcast_to((W_beta[D:2 * D] - W_beta[2 * D:3 * D]).reshape(1, D), (P, D)).astype(np.float32).copy(),
        ident=np.eye(P, dtype=np.float32),
        ones4=np.ones((P, 4), BF16),
    )

    nembT = np.ascontiguousarray(nemb.T)    # [128, N]

    in_maps = []
    unperm = []
    for c in range(NCORES):
        pc = per_core[c]
        perm = pc["edge_perm"]
        safe = np.where(perm >= 0, perm, 0)

        # streams in [feat, edge] layout, bf16
        embsT = np.ascontiguousarray(nembT[:, src[safe]]).astype(BF16)
        embdT = np.ascontiguousarray(nembT[:, dst[safe]]).astype(BF16)
        bondT = np.ascontiguousarray(bond[safe].T).astype(BF16)

        # per-tile-partition arrays [128, TT]: element (p, t) = edge t*128+p
        ba_pm = np.ascontiguousarray(
            np.where(perm >= 0, ba[safe], 0.0).astype(np.float32).reshape(TT, P).T)
        dloc = pc["dst_local"].astype(np.int32).reshape(TT, P).T  # [128, TT]
        # local_scatter indices: within chunk, column = tile_in_chunk*128 + dst_local
        sidx = np.full((P, TT), -1, np.int32)
        off = 0
        for j in range(NBLK):
            for ch in chunks[j]:
                for t in range(ch):
                    col = off + t
                    d_ = dloc[:, col]
                    sidx[:, col] = np.where(d_ >= 0, t * P + d_, -1)
                off += ch
        sidx = sidx.astype(np.int16)

        # node-side: local x in [128, NBLK*128] partition-major by block,
        # following block_order (position j holds original block order[j])
        nx = np.zeros((P, NBLK * P), np.float32)
        base = c * NPC
        for j, bj in enumerate(pc["block_order"]):
            lo = base + bj * P
            hi = min(lo + P, base + NPC)
            n = hi - lo
            nx[:n, j * P:j * P + P] = nemb[lo:hi, :]

        in_maps.append(dict(
            embsT=embsT, embdT=embdT, bondT=bondT,
            ba=ba_pm, sidx=sidx, nx=nx, **consts))
        unperm.append(pc["block_order"])

    return T_sched, chunks, TT, in_maps, unperm


# --------------------------------------------------------------------------
# device program
# --------------------------------------------------------------------------

def build_program(T_sched, chunks, TT):
    # Pin a single ACT function table covering every func we use (exp, ln,
    # copy, relu, identity, square) so no mid-kernel table reloads (~1.3us
    # each) are emitted.
    _orig_tables = bacc.get_activation_tables
    def _one_table(arch):
        tabs = _orig_tables(arch)
        name = "natural_log_exp_and_others"
        return {name: tabs[name]} if name in tabs else tabs
    bacc.get_activation_tables = _one_table
    try:
        return _build_program_inner(T_sched, chunks, TT)
    finally:
        bacc.get_activation_tables = _orig_tables


def _build_program_inner(T_sched, chunks, TT):
    nc = bacc.Bacc("TRN2", target_bir_lowering=False, debug=False,
                   num_devices=NCORES)
    f32 = mybir.dt.float32
    bf16 = mybir.dt.bfloat16
    i16 = mybir.dt.int16
    AL = mybir.AluOpType
    AF = mybir.ActivationFunctionType

    embsT_in = nc.dram_tensor("embsT", [P, TT * P], bf16, kind="ExternalInput")
    embdT_in = nc.dram_tensor("embdT", [P, TT * P], bf16, kind="ExternalInput")
    bondT_in = nc.dram_tensor("bondT", [P, TT * P], bf16, kind="ExternalInput")
    ba_in = nc.dram_tensor("ba", [P, TT], f32, kind="ExternalInput")
    sidx_in = nc.dram_tensor("sidx", [P, TT], i16, kind="ExternalInput")
    nx_in = nc.dram_tensor("nx", [P, NBLK * P], f32, kind="ExternalInput")
    WkT_in = nc.dram_tensor("WkT", [P, P], bf16, kind="ExternalInput")
    WqT_in = nc.dram_tensor("WqT", [P, P], bf16, kind="ExternalInput")
    WvT_in = nc.dram_tensor("WvT", [P, P], bf16, kind="ExternalInput")
    W1T_in = nc.dram_tensor("W1T", [P, D_FF], bf16, kind="ExternalInput")
    W2T_in = nc.dram_tensor("W2T", [D_FF, P], bf16, kind="ExternalInput")
    wdis_in = nc.dram_tensor("wdis", [P, H], f32, kind="ExternalInput")
    wbh_in = nc.dram_tensor("wbh", [P, D], f32, kind="ExternalInput")
    wbx_in = nc.dram_tensor("wbx", [P, D], f32, kind="ExternalInput")
    ident_in = nc.dram_tensor("ident", [P, P], f32, kind="ExternalInput")
    ones4_in = nc.dram_tensor("ones4", [P, 4], bf16, kind="ExternalInput")

    out_dram = nc.dram_tensor("out", [P, NBLK * P], f32, kind="ExternalOutput")

    with ExitStack() as ctx:
        tc = ctx.enter_context(tile.TileContext(nc))
        cst = ctx.enter_context(tc.tile_pool(name="cst", bufs=1))
        res = ctx.enter_context(tc.tile_pool(name="res", bufs=1))
        edg = ctx.enter_context(tc.tile_pool(name="edg", bufs=3))
        sml = ctx.enter_context(tc.tile_pool(name="sml", bufs=4))
        wrk = ctx.enter_context(tc.tile_pool(name="wrk", bufs=3))
        epi = ctx.enter_context(tc.tile_pool(name="epi", bufs=3))
        psMM = ctx.enter_context(tc.tile_pool(name="psMM", bufs=4, space="PSUM"))
        psFFN = ctx.enter_context(tc.tile_pool(name="psFFN", bufs=2, space="PSUM"))
        psACC = ctx.enter_context(tc.tile_pool(name="psACC", bufs=2, space="PSUM"))

        def load_const(inp, shape, dtype, tag):
            t = cst.tile(shape, dtype, tag=tag)
            nc.sync.dma_start(out=t[:], in_=inp[:, :])
            return t

        WkT = load_const(WkT_in, [P, P], bf16, "WkT")
        WqT = load_const(WqT_in, [P, P], bf16, "WqT")
        WvT = load_const(WvT_in, [P, P], bf16, "WvT")
        W1T = load_const(W1T_in, [P, D_FF], bf16, "W1T")
        wdis = load_const(wdis_in, [P, H], f32, "wdis")
        wbh = load_const(wbh_in, [P, D], f32, "wbh")
        wbx = load_const(wbx_in, [P, D], f32, "wbx")
        ident = load_const(ident_in, [P, P], f32, "ident")
        ones4 = load_const(ones4_in, [P, 4], bf16, "ones4")

        eps_t = cst.tile([P, 1], f32, tag="eps")
        nc.gpsimd.memset(eps_t[:], EPS)

        W2Ta = cst.tile([P, P], bf16, tag="W2Ta")
        nc.sync.dma_start(out=W2Ta[:], in_=W2T_in[0:P, :])
        W2Tb = cst.tile([P, P], bf16, tag="W2Tb")
        nc.sync.dma_start(out=W2Tb[:], in_=W2T_in[P:2 * P, :])

        nx_res = res.tile([P, NBLK * P], f32)
        nc.sync.dma_start(out=nx_res[:], in_=nx_in[:, :])

        off = 0
        for j in range(NBLK):
            T = T_sched[j]
            e0 = off * P

            bond_t = edg.tile([P, T * P], bf16, tag="bond")
            nc.sync.dma_start(out=bond_t[:], in_=bondT_in[:, e0:e0 + T * P])
            embs_t = edg.tile([P, T * P], bf16, tag="embs")
            nc.sync.dma_start(out=embs_t[:], in_=embsT_in[:, e0:e0 + T * P])
            embd_t = edg.tile([P, T * P], bf16, tag="embd")
            nc.sync.dma_start(out=embd_t[:], in_=embdT_in[:, e0:e0 + T * P])
            ba_t = sml.tile([P, T], f32, tag="ba")
            nc.sync.dma_start(out=ba_t[:], in_=ba_in[:, off:off + T])
            sidx_t = sml.tile([P, T], i16, tag="sidx")
            nc.sync.dma_start(out=sidx_t[:], in_=sidx_in[:, off:off + T])

            att_t = sml.tile([P, T * H], f32, tag="att")
            wvp_t = wrk.tile([P, T * W136], bf16, tag="wvp")

            # bias = ba*wdis (no dep on att; runs early on Pool)
            bias_t = sml.tile([P, T * H], f32, tag="bias")
            nc.gpsimd.tensor_tensor(
                out=bias_t[:].rearrange("p (t h) -> p t h", h=H),
                in0=ba_t[:].unsqueeze(-1).to_broadcast([P, T, H]),
                in1=wdis[:].unsqueeze(1).to_broadcast([P, T, H]),
                op=AL.mult)

            # pass 1: attention logits
            c0 = 0
            for cw in chunks[j]:
                q_ps = psMM.tile([P, cw * P], f32, tag="mm")
                for t in range(cw):
                    nc.tensor.matmul(
                        out=q_ps[:, t * P:(t + 1) * P],
                        lhsT=embd_t[:, (c0 + t) * P:(c0 + t + 1) * P],
                        rhs=WqT[:], start=True, stop=True)
                q_sb = wrk.tile([P, cw * P], bf16, tag="qsb")
                nc.scalar.copy(out=q_sb[:], in_=q_ps[:])
                k_ps = psMM.tile([P, cw * P], f32, tag="mm")
                for t in range(cw):
                    nc.tensor.matmul(
                        out=k_ps[:, t * P:(t + 1) * P],
                        lhsT=embs_t[:, (c0 + t) * P:(c0 + t + 1) * P],
                        rhs=WkT[:], start=True, stop=True)
                kq_t = wrk.tile([P, cw * P], bf16, tag="kq")
                nc.vector.tensor_tensor(out=kq_t[:], in0=k_ps[:], in1=q_sb[:],
                                        op=AL.mult)
                nc.vector.tensor_reduce(
                    out=att_t[:].rearrange("p (t h) -> p t h", h=H)[:, c0:c0 + cw, :],
                    in_=kq_t[:].rearrange("p (t h d) -> p t h d", h=H, d=DH),
                    axis=mybir.AxisListType.X, op=AL.add)
                c0 += cw

            # logits -> p, written into the p-slots of wvp
            nc.vector.tensor_tensor(out=bias_t[:], in0=bias_t[:], in1=att_t[:],
                                    op=AL.add)
            p_t = sml.tile([P, T * H], bf16, tag="pexp")
            nc.scalar.activation(out=p_t[:], in_=bias_t[:], func=AF.Exp)
            nc.vector.tensor_copy(
                out=wvp_t[:].rearrange("p (t w) -> p t w", w=W136)[:, :, D:W136],
                in_=p_t[:].rearrange("p (t h) -> p t h", h=H))

            # pass 2: v, wv, one-hot scatter
            acc_ps = psACC.tile([P, 512], f32, tag="acc")
            c0 = 0
            for cw in chunks[j]:
                v_ps = psMM.tile([P, cw * P], f32, tag="mm")
                for t in range(cw):
                    nc.tensor.matmul(
                        out=v_ps[:, t * P:(t + 1) * P],
                        lhsT=bond_t[:, (c0 + t) * P:(c0 + t + 1) * P],
                        rhs=WvT[:], start=True, stop=True)
                s_t = wrk.tile([P, cw * P], bf16, tag="sh")
                nc.gpsimd.local_scatter(
                    out_ap=s_t[:], data_ap=ones4[:, 0:cw],
                    idxs_ap=sidx_t[:, c0:c0 + cw],
                    channels=P, num_elems=cw * P, num_idxs=cw)
                nc.vector.tensor_tensor(
                    out=wvp_t[:].rearrange("p (t w) -> p t w", w=W136)
                        [:, c0:c0 + cw, 0:D].rearrange("p t (h d) -> p t h d", h=H),
                    in0=v_ps[:].rearrange("p (t h d) -> p t h d", h=H, d=DH),
                    in1=wvp_t[:].rearrange("p (t w) -> p t w", w=W136)
                        [:, c0:c0 + cw, D:W136].unsqueeze(-1).to_broadcast([P, cw, H, DH]),
                    op=AL.mult)
                for t in range(cw):
                    gt = c0 + t
                    nc.tensor.matmul(
                        out=acc_ps[:, 0:W136],
                        lhsT=s_t[:, t * P:(t + 1) * P],
                        rhs=wvp_t[:, gt * W136:(gt + 1) * W136],
                        start=(gt == 0), stop=(gt == T - 1))
                c0 += cw

            # ---- per-block node epilogue (overlaps with later blocks) ----
            nxs = nx_res[:, j * P:(j + 1) * P]
            acc_t = epi.tile([P, W136], f32, tag="acct")
            nc.scalar.copy(out=acc_t[:], in_=acc_ps[:, 0:W136])

            rs_t = sml.tile([P, H], f32, tag="rs")
            nc.gpsimd.tensor_scalar(out=rs_t[:], in0=acc_t[:, D:W136],
                                    scalar1=SEPS, scalar2=None, op0=AL.add)
            nc.vector.reciprocal(out=rs_t[:], in_=rs_t[:])
            he_t = epi.tile([P, P], f32, tag="he")
            nc.gpsimd.tensor_tensor(
                out=he_t[:].rearrange("p (h d) -> p h d", h=H),
                in0=acc_t[:, 0:D].rearrange("p (h d) -> p h d", h=H),
                in1=rs_t[:].unsqueeze(-1).to_broadcast([P, H, DH]),
                op=AL.mult)

            # beta = sigmoid(he.wbh + x.wbx)
            z_t = sml.tile([P, 4], f32, tag="z")
            scr1 = epi.tile([P, P], f32, tag="scr1")
            nc.gpsimd.scalar_tensor_tensor(
                out=scr1[:], in0=he_t[:], scalar=1.0, in1=wbh[:],
                op0=AL.mult, op1=AL.mult, accum_out=z_t[:, 0:1])
            scr2 = epi.tile([P, P], f32, tag="scr2")
            nc.gpsimd.scalar_tensor_tensor(
                out=scr2[:], in0=nxs, scalar=1.0, in1=wbx[:],
                op0=AL.mult, op1=AL.mult, accum_out=z_t[:, 1:2])
            nc.gpsimd.tensor_tensor(out=z_t[:, 2:3], in0=z_t[:, 0:1],
                                    in1=z_t[:, 1:2], op=AL.add)
            beta_t = sml.tile([P, 1], f32, tag="beta")
            nc.scalar.activation(out=beta_t[:], in_=z_t[:, 2:3], func=AF.Exp,
                                 scale=-1.0)
            nc.gpsimd.tensor_scalar(out=beta_t[:], in0=beta_t[:], scalar1=1.0,
                                    scalar2=None, op0=AL.add)
            nc.vector.reciprocal(out=beta_t[:], in_=beta_t[:])

            # he2 = he + beta*(x - he)
            d_t = epi.tile([P, P], f32, tag="d")
            nc.gpsimd.tensor_tensor(out=d_t[:], in0=nxs, in1=he_t[:],
                                    op=AL.subtract)
            he2_t = epi.tile([P, P], f32, tag="he2")
            nc.gpsimd.scalar_tensor_tensor(
                out=he2_t[:], in0=d_t[:], scalar=beta_t[:, 0:1], in1=he_t[:],
                op0=AL.mult, op1=AL.add)

            def layer_norm(src_t, dst_tag, center_eng):
                """dst = LN(src) for one block; returns dst tile."""
                negmu = sml.tile([P, 1], f32, tag="negmu")
                nc.vector.tensor_reduce(out=negmu[:], in_=src_t[:],
                                        axis=mybir.AxisListType.X, op=AL.add,
                                        negate=True)
                nc.vector.tensor_scalar(out=negmu[:], in0=negmu[:],
                                        scalar1=1.0 / P, scalar2=None,
                                        op0=AL.mult)
                hc_t = epi.tile([P, P], f32, tag=dst_tag + "hc")
                nc.gpsimd.tensor_tensor(out=hc_t[:], in0=src_t[:],
                                        in1=negmu[:, 0:1].to_broadcast([P, P]),
                                        op=AL.add)
                sq_t = epi.tile([P, P], f32, tag=dst_tag + "sq")
                var_t = sml.tile([P, 1], f32, tag="var")
                nc.gpsimd.scalar_tensor_tensor(
                    out=sq_t[:], in0=hc_t[:], scalar=1.0, in1=hc_t[:],
                    op0=AL.mult, op1=AL.mult, accum_out=var_t[:])
                nc.gpsimd.tensor_scalar(out=var_t[:], in0=var_t[:],
                                        scalar1=1.0 / P, scalar2=EPS,
                                        op0=AL.mult, op1=AL.add)
                nc.scalar.activation(out=var_t[:], in_=var_t[:], func=AF.Ln)
                nc.scalar.activation(out=var_t[:], in_=var_t[:], func=AF.Exp,
                                     scale=-0.5)
                y_t = epi.tile([P, P], f32, tag=dst_tag)
                nc.vector.tensor_scalar(out=y_t[:], in0=hc_t[:],
                                        scalar1=var_t[:, 0:1], scalar2=None,
                                        op0=AL.mult)
                return y_t

            y_t = layer_norm(he2_t, "y", None)

            yT_ps = psFFN.tile([P, P], f32, tag="ffn")
            nc.tensor.transpose(out=yT_ps[:], in_=y_t[:], identity=ident[:])
            yT_sb = epi.tile([P, P], bf16, tag="yts")
            nc.scalar.copy(out=yT_sb[:], in_=yT_ps[:])
            h1a_ps = psFFN.tile([P, P], f32, tag="ffn")
            nc.tensor.matmul(out=h1a_ps[:], lhsT=W1T[:, 0:P], rhs=yT_sb[:],
                             start=True, stop=True)
            h1b_ps = psFFN.tile([P, P], f32, tag="ffn")
            nc.tensor.matmul(out=h1b_ps[:], lhsT=W1T[:, P:2 * P], rhs=yT_sb[:],
                             start=True, stop=True)
            h1a_sb = epi.tile([P, P], bf16, tag="h1as")
            nc.scalar.activation(out=h1a_sb[:], in_=h1a_ps[:], func=AF.Relu)
            h1b_sb = epi.tile([P, P], bf16, tag="h1bs")
            nc.scalar.activation(out=h1b_sb[:], in_=h1b_ps[:], func=AF.Relu)
            h2_ps = psFFN.tile([P, P], f32, tag="ffn")
            nc.tensor.matmul(out=h2_ps[:], lhsT=h1a_sb[:], rhs=W2Ta[:],
                             start=True, stop=False)
            nc.tensor.matmul(out=h2_ps[:], lhsT=h1b_sb[:], rhs=W2Tb[:],
                             start=False, stop=True)
            o_t = epi.tile([P, P], f32, tag="o")
            nc.vector.tensor_tensor(out=o_t[:], in0=h2_ps[:], in1=he2_t[:],
                                    op=AL.add)

            out_t = layer_norm(o_t, "out", None)
            nc.sync.dma_start(out=out_dram[:, j * P:(j + 1) * P], in_=out_t[:])

            off += T

    nc.finalize()
    return nc


# --------------------------------------------------------------------------
# entry point
# --------------------------------------------------------------------------

def kernel(**inputs):
    import os
    T_sched, chunks, TT, in_maps, block_orders = _host_prep(inputs)

    key = tuple(T_sched)
    if key not in _CACHE:
        _CACHE[key] = build_program(T_sched, chunks, TT)
    nc = _CACHE[key]

    trace = bool(os.environ.get("BASS_KERNEL_TRACE"))
    tmpdir = os.environ.get("BASS_KERNEL_TRACE_DIR") or None
    results = run_bass_kernel_spmd(nc, in_maps, core_ids=list(range(NCORES)),
                                   trace=trace, tmpdir=tmpdir)
    if trace and results.exec_time_ns is not None:
        print(f"HW exec time: {results.exec_time_ns} ns")

    out = np.zeros((N_NODES, D), np.float32)
    for c in range(NCORES):
        o = results.results[c]["out"]          # [128, NBLK*128]
        base = c * NPC
        for j, bj in enumerate(block_orders[c]):
            lo = base + bj * P
            hi = min(lo + P, base + NPC)
            n = hi - lo
            out[lo:hi, :] = o[:n, j * P:j * P + P]
    return out


# revision 18
# speedup vs baseline: 1.5192x; 1.0198x over previous
"""Trainium2 Bass kernel for nn_Bond2AtomLayer (GNN message passing).

Strategy (8-core SPMD, dst-node partitioned):
- Host: sort edges by dst, partition nodes into 8 ranges of 6250; each core
  owns the edges whose dst falls in its range. Within a core, edges are
  grouped into 128-node "blocks" (49 per core), each block's edges padded to
  a whole number of 128-edge tiles. Blocks are assigned to fixed program
  positions with a global per-position tile count (max over cores) so all
  8 cores run one identical program.
- Host pre-gathers node_emb[src], node_emb[dst] and bond rows into per-core
  edge-order streams, transposed to [feat, edge] (bf16) so the device needs
  no gather at all — k/q/v are computed per 128-edge tile by PE matmuls
  against the small weight matrices.
- Edge softmax: att[e,h] = sum_d k[e,hd]*q[e,hd] (DVE mult + grouped reduce),
  logits = att/4 + ba*W_dis (1/4 folded into Wq), p = exp(logits) without
  max-subtraction (logits are O(1); softmax is shift-invariant so this
  matches the reference numerically).
- Scatter-sum: per tile a one-hot S[e,n] (bf16, built by GPSIMD local_scatter)
  and one PE matmul accumulates [wv | p] into PSUM per 128-node block;
  ft = wsum / s, then beta-gating, LN, FFN, LN on-chip (node-major, batched).
"""
import sys

sys.path.insert(0, "/opt/trn_rl_repo")

import numpy as np
import ml_dtypes
from contextlib import ExitStack

import concourse.bass as bass
import concourse.tile as tile
from concourse import bacc, mybir
from concourse.bass_utils import run_bass_kernel_spmd

BF16 = ml_dtypes.bfloat16

N_NODES = 50000
N_EDGES = 800000
D = 128
H = 8
DH = 16
D_FF = 256
P = 128
NCORES = 8
NPC = N_NODES // NCORES        # 6250 nodes per core
NBLK = (NPC + P - 1) // P      # 49 blocks per core (last has 106 nodes)
NPAD = NBLK * P                # 6272
EPS = 1e-5
SEPS = 1e-30                   # guard for 1/s on isolated nodes
W136 = D + H                   # 136: [wv | p] scatter payload width

_CACHE = {}


# --------------------------------------------------------------------------
# host-side scheduling
# --------------------------------------------------------------------------

def _schedule(dst):
    """Partition edges by dst; build per-core block schedules.

    Returns:
      T_sched: list of per-position tile counts (same for all cores)
      chunks:  list of chunk widths per position (even, <=4 each)
      per_core: list of dicts with keys:
        edge_perm [TT*128] int64 (index into full edge list; -1 = pad)
        dst_local [TT*128] int16 (node index within block; -1 = pad)
        block_order: list of per-position original block ids
    """
    E = dst.shape[0]
    core = dst // NPC
    blk = (dst % NPC) // P
    nloc = (dst % NPC) % P

    order = np.argsort(dst, kind="stable")

    per_core_raw = []
    counts = np.zeros((NCORES, NBLK), np.int64)
    for c in range(NCORES):
        sel = order[(core[order] == c)]
        b = blk[sel]
        blists = []
        for j in range(NBLK):
            eb = sel[b == j]
            blists.append(eb)
            counts[c, j] = len(eb)
        per_core_raw.append(blists)

    tiles = (counts + P - 1) // P           # [NCORES, NBLK]
    tiles = np.maximum(tiles, 1)
    # sort each core's blocks by tile count desc; per-position count = max
    orders = [list(np.argsort(-tiles[c], kind="stable")) for c in range(NCORES)]
    T_sched = []
    for j in range(NBLK):
        t = max(tiles[c, orders[c][j]] for c in range(NCORES))
        t = int(t + (t & 1))                # round up to even
        T_sched.append(max(t, 2))

    chunks = []
    for t in T_sched:
        ch = [4] * (t // 4)
        if t % 4:
            ch.append(t % 4)                # t even => remainder 2
        chunks.append(ch)

    TT = sum(T_sched)
    per_core = []
    for c in range(NCORES):
        perm = np.full(TT * P, -1, np.int64)
        dloc = np.full(TT * P, -1, np.int16)
        off = 0
        for j in range(NBLK):
            bj = orders[c][j]
            eb = per_core_raw[c][bj]
            perm[off:off + len(eb)] = eb
            dloc[off:off + len(eb)] = nloc[eb].astype(np.int16)
            off += T_sched[j] * P
        per_core.append(dict(edge_perm=perm, dst_local=dloc, block_order=orders[c]))
    return T_sched, chunks, TT, per_core


def _host_prep(inputs):
    bond = np.asarray(inputs["bond_embedding"], np.float32)
    nemb = np.asarray(inputs["node_embedding"], np.float32)
    ba = np.asarray(inputs["basic_attn"], np.float32).reshape(-1)
    Wk = np.asarray(inputs["Wk"], np.float32)
    Wq = np.asarray(inputs["Wq"], np.float32)
    Wv = np.asarray(inputs["Wv"], np.float32)
    W_dis = np.asarray(inputs["W_dis"], np.float32)
    W_beta = np.asarray(inputs["W_beta"], np.float32).reshape(-1)
    W1 = np.asarray(inputs["W1"], np.float32)
    W2 = np.asarray(inputs["W2"], np.float32)
    src = np.asarray(inputs["src"], np.int64)
    dst = np.asarray(inputs["dst"], np.int64)

    T_sched, chunks, TT, per_core = _schedule(dst)

    # weights (replicated)
    scale = 1.0 / np.sqrt(np.float32(DH))
    consts = dict(
        WkT=np.ascontiguousarray(Wk.T).astype(BF16),
        WqT=np.ascontiguousarray((Wq * scale).T).astype(BF16),
        WvT=np.ascontiguousarray(Wv.T).astype(BF16),
        W1T=np.ascontiguousarray(W1.T).astype(BF16),        # [128, 256]
        W2T=np.ascontiguousarray(W2.T).astype(BF16),        # [256, 128]
        wdis=np.broadcast_to(W_dis.reshape(1, H), (P, H)).astype(np.float32).copy(),
        wbh=np.broadcast_to((W_beta[0:D] + W_beta[2 * D:3 * D]).reshape(1, D), (P, D)).astype(np.float32).copy(),
        wbx=np.broadcast_to((W_beta[D:2 * D] - W_beta[2 * D:3 * D]).reshape(1, D), (P, D)).astype(np.float32).copy(),
        ident=np.eye(P, dtype=np.float32),
        ones4=np.ones((P, 4), BF16),
    )

    nembT = np.ascontiguousarray(nemb.T)    # [128, N]

    in_maps = []
    unperm = []
    for c in range(NCORES):
        pc = per_core[c]
        perm = pc["edge_perm"]
        safe = np.where(perm >= 0, perm, 0)

        # streams in [feat, edge] layout, bf16
        embsT = np.ascontiguousarray(nembT[:, src[safe]]).astype(BF16)
        embdT = np.ascontiguousarray(nembT[:, dst[safe]]).astype(BF16)
        bondT = np.ascontiguousarray(bond[safe].T).astype(BF16)

        # per-tile-partition arrays [128, TT]: element (p, t) = edge t*128+p
        ba_pm = np.ascontiguousarray(
            np.where(perm >= 0, ba[safe], 0.0).astype(np.float32).reshape(TT, P).T)
        dloc = pc["dst_local"].astype(np.int32).reshape(TT, P).T  # [128, TT]
        # local_scatter indices: within chunk, column = tile_in_chunk*128 + dst_local
        sidx = np.full((P, TT), -1, np.int32)
        off = 0
        for j in range(NBLK):
            for ch in chunks[j]:
                for t in range(ch):
                    col = off + t
                    d_ = dloc[:, col]
                    sidx[:, col] = np.where(d_ >= 0, t * P + d_, -1)
                off += ch
        sidx = sidx.astype(np.int16)

        # node-side: local x in [128, NBLK*128] partition-major by block,
        # following block_order (position j holds original block order[j])
        nx = np.zeros((P, NBLK * P), np.float32)
        base = c * NPC
        for j, bj in enumerate(pc["block_order"]):
            lo = base + bj * P
            hi = min(lo + P, base + NPC)
            n = hi - lo
            nx[:n, j * P:j * P + P] = nemb[lo:hi, :]

        in_maps.append(dict(
            embsT=embsT, embdT=embdT, bondT=bondT,
            ba=ba_pm, sidx=sidx, nx=nx, **consts))
        unperm.append(pc["block_order"])

    return T_sched, chunks, TT, in_maps, unperm


# --------------------------------------------------------------------------
# device program
# --------------------------------------------------------------------------

def build_program(T_sched, chunks, TT):
    # Pin a single ACT function table covering every func we use (exp, ln,
    # copy, relu, identity, square) so no mid-kernel table reloads (~1.3us
    # each) are emitted.
    _orig_tables = bacc.get_activation_tables
    def _one_table(arch):
        tabs = _orig_tables(arch)
        name = "natural_log_exp_and_others"
        return {name: tabs[name]} if name in tabs else tabs
    bacc.get_activation_tables = _one_table
    try:
        return _build_program_inner(T_sched, chunks, TT)
    finally:
        bacc.get_activation_tables = _orig_tables


def _build_program_inner(T_sched, chunks, TT):
    nc = bacc.Bacc("TRN2", target_bir_lowering=False, debug=False,
                   num_devices=NCORES)
    f32 = mybir.dt.float32
    bf16 = mybir.dt.bfloat16
    i16 = mybir.dt.int16
    AL = mybir.AluOpType
    AF = mybir.ActivationFunctionType

    embsT_in = nc.dram_tensor("embsT", [P, TT * P], bf16, kind="ExternalInput")
    embdT_in = nc.dram_tensor("embdT", [P, TT * P], bf16, kind="ExternalInput")
    bondT_in = nc.dram_tensor("bondT", [P, TT * P], bf16, kind="ExternalInput")
    ba_in = nc.dram_tensor("ba", [P, TT], f32, kind="ExternalInput")
    sidx_in = nc.dram_tensor("sidx", [P, TT], i16, kind="ExternalInput")
    nx_in = nc.dram_tensor("nx", [P, NBLK * P], f32, kind="ExternalInput")
    WkT_in = nc.dram_tensor("WkT", [P, P], bf16, kind="ExternalInput")
    WqT_in = nc.dram_tensor("WqT", [P, P], bf16, kind="ExternalInput")
    WvT_in = nc.dram_tensor("WvT", [P, P], bf16, kind="ExternalInput")
    W1T_in = nc.dram_tensor("W1T", [P, D_FF], bf16, kind="ExternalInput")
    W2T_in = nc.dram_tensor("W2T", [D_FF, P], bf16, kind="ExternalInput")
    wdis_in = nc.dram_tensor("wdis", [P, H], f32, kind="ExternalInput")
    wbh_in = nc.dram_tensor("wbh", [P, D], f32, kind="ExternalInput")
    wbx_in = nc.dram_tensor("wbx", [P, D], f32, kind="ExternalInput")
    ident_in = nc.dram_tensor("ident", [P, P], f32, kind="ExternalInput")
    ones4_in = nc.dram_tensor("ones4", [P, 4], bf16, kind="ExternalInput")

    out_dram = nc.dram_tensor("out", [P, NBLK * P], f32, kind="ExternalOutput")

    with ExitStack() as ctx:
        tc = ctx.enter_context(tile.TileContext(nc))
        cst = ctx.enter_context(tc.tile_pool(name="cst", bufs=1))
        res = ctx.enter_context(tc.tile_pool(name="res", bufs=1))
        edg = ctx.enter_context(tc.tile_pool(name="edg", bufs=4))
        sml = ctx.enter_context(tc.tile_pool(name="sml", bufs=4))
        wrk = ctx.enter_context(tc.tile_pool(name="wrk", bufs=3))
        epi = ctx.enter_context(tc.tile_pool(name="epi", bufs=3))
        psMM = ctx.enter_context(tc.tile_pool(name="psMM", bufs=5, space="PSUM"))
        psFFN = ctx.enter_context(tc.tile_pool(name="psFFN", bufs=1, space="PSUM"))
        psACC = ctx.enter_context(tc.tile_pool(name="psACC", bufs=2, space="PSUM"))

        def load_const(inp, shape, dtype, tag):
            t = cst.tile(shape, dtype, tag=tag)
            nc.sync.dma_start(out=t[:], in_=inp[:, :])
            return t

        WkT = load_const(WkT_in, [P, P], bf16, "WkT")
        WqT = load_const(WqT_in, [P, P], bf16, "WqT")
        WvT = load_const(WvT_in, [P, P], bf16, "WvT")
        W1T = load_const(W1T_in, [P, D_FF], bf16, "W1T")
        wdis = load_const(wdis_in, [P, H], f32, "wdis")
        wbh = load_const(wbh_in, [P, D], f32, "wbh")
        wbx = load_const(wbx_in, [P, D], f32, "wbx")
        ident = load_const(ident_in, [P, P], f32, "ident")
        ones4 = load_const(ones4_in, [P, 4], bf16, "ones4")

        eps_t = cst.tile([P, 1], f32, tag="eps")
        nc.gpsimd.memset(eps_t[:], EPS)

        W2Ta = cst.tile([P, P], bf16, tag="W2Ta")
        nc.sync.dma_start(out=W2Ta[:], in_=W2T_in[0:P, :])
        W2Tb = cst.tile([P, P], bf16, tag="W2Tb")
        nc.sync.dma_start(out=W2Tb[:], in_=W2T_in[P:2 * P, :])

        nx_res = res.tile([P, NBLK * P], f32)
        nc.sync.dma_start(out=nx_res[:], in_=nx_in[:, :])

        off = 0
        for j in range(NBLK):
            T = T_sched[j]
            e0 = off * P

            bond_t = edg.tile([P, T * P], bf16, tag="bond")
            nc.sync.dma_start(out=bond_t[:], in_=bondT_in[:, e0:e0 + T * P])
            embs_t = edg.tile([P, T * P], bf16, tag="embs")
            nc.sync.dma_start(out=embs_t[:], in_=embsT_in[:, e0:e0 + T * P])
            embd_t = edg.tile([P, T * P], bf16, tag="embd")
            nc.sync.dma_start(out=embd_t[:], in_=embdT_in[:, e0:e0 + T * P])
            ba_t = sml.tile([P, T], f32, tag="ba")
            nc.sync.dma_start(out=ba_t[:], in_=ba_in[:, off:off + T])
            sidx_t = sml.tile([P, T], i16, tag="sidx")
            nc.sync.dma_start(out=sidx_t[:], in_=sidx_in[:, off:off + T])

            att_t = sml.tile([P, T * H], f32, tag="att")
            wvp_t = wrk.tile([P, T * W136], bf16, tag="wvp")

            # bias = ba*wdis (no dep on att; runs early on Pool)
            bias_t = sml.tile([P, T * H], f32, tag="bias")
            nc.gpsimd.tensor_tensor(
                out=bias_t[:].rearrange("p (t h) -> p t h", h=H),
                in0=ba_t[:].unsqueeze(-1).to_broadcast([P, T, H]),
                in1=wdis[:].unsqueeze(1).to_broadcast([P, T, H]),
                op=AL.mult)

            # pass 1: attention logits
            c0 = 0
            for cw in chunks[j]:
                q_ps = psMM.tile([P, cw * P], f32, tag="mm")
                for t in range(cw):
                    nc.tensor.matmul(
                        out=q_ps[:, t * P:(t + 1) * P],
                        lhsT=embd_t[:, (c0 + t) * P:(c0 + t + 1) * P],
                        rhs=WqT[:], start=True, stop=True)
                q_sb = wrk.tile([P, cw * P], bf16, tag="qsb")
                nc.scalar.copy(out=q_sb[:], in_=q_ps[:])
                k_ps = psMM.tile([P, cw * P], f32, tag="mm")
                for t in range(cw):
                    nc.tensor.matmul(
                        out=k_ps[:, t * P:(t + 1) * P],
                        lhsT=embs_t[:, (c0 + t) * P:(c0 + t + 1) * P],
                        rhs=WkT[:], start=True, stop=True)
                kq_t = wrk.tile([P, cw * P], bf16, tag="kq")
                nc.vector.tensor_tensor(out=kq_t[:], in0=k_ps[:], in1=q_sb[:],
                                        op=AL.mult)
                nc.vector.tensor_reduce(
                    out=att_t[:].rearrange("p (t h) -> p t h", h=H)[:, c0:c0 + cw, :],
                    in_=kq_t[:].rearrange("p (t h d) -> p t h d", h=H, d=DH),
                    axis=mybir.AxisListType.X, op=AL.add)
                c0 += cw

            # logits -> p, written into the p-slots of wvp
            nc.vector.tensor_tensor(out=bias_t[:], in0=bias_t[:], in1=att_t[:],
                                    op=AL.add)
            p_t = sml.tile([P, T * H], bf16, tag="pexp")
            nc.scalar.activation(out=p_t[:], in_=bias_t[:], func=AF.Exp)
            nc.vector.tensor_copy(
                out=wvp_t[:].rearrange("p (t w) -> p t w", w=W136)[:, :, D:W136],
                in_=p_t[:].rearrange("p (t h) -> p t h", h=H))

            # pass 2: v, wv, one-hot scatter
            acc_ps = psACC.tile([P, 512], f32, tag="acc")
            c0 = 0
            for cw in chunks[j]:
                v_ps = psMM.tile([P, cw * P], f32, tag="mm")
                for t in range(cw):
                    nc.tensor.matmul(
                        out=v_ps[:, t * P:(t + 1) * P],
                        lhsT=bond_t[:, (c0 + t) * P:(c0 + t + 1) * P],
                        rhs=WvT[:], start=True, stop=True)
                s_t = wrk.tile([P, cw * P], bf16, tag="sh")
                nc.gpsimd.local_scatter(
                    out_ap=s_t[:], data_ap=ones4[:, 0:cw],
                    idxs_ap=sidx_t[:, c0:c0 + cw],
                    channels=P, num_elems=cw * P, num_idxs=cw)
                nc.vector.tensor_tensor(
                    out=wvp_t[:].rearrange("p (t w) -> p t w", w=W136)
                        [:, c0:c0 + cw, 0:D].rearrange("p t (h d) -> p t h d", h=H),
                    in0=v_ps[:].rearrange("p (t h d) -> p t h d", h=H, d=DH),
                    in1=wvp_t[:].rearrange("p (t w) -> p t w", w=W136)
                        [:, c0:c0 + cw, D:W136].unsqueeze(-1).to_broadcast([P, cw, H, DH]),
                    op=AL.mult)
                for t in range(cw):
                    gt = c0 + t
                    nc.tensor.matmul(
                        out=acc_ps[:, 0:W136],
                        lhsT=s_t[:, t * P:(t + 1) * P],
                        rhs=wvp_t[:, gt * W136:(gt + 1) * W136],
                        start=(gt == 0), stop=(gt == T - 1))
                c0 += cw

            # ---- per-block node epilogue (overlaps with later blocks) ----
            nxs = nx_res[:, j * P:(j + 1) * P]
            acc_t = epi.tile([P, W136], f32, tag="acct")
            nc.scalar.copy(out=acc_t[:], in_=acc_ps[:, 0:W136])

            rs_t = sml.tile([P, H], f32, tag="rs")
            nc.gpsimd.tensor_scalar(out=rs_t[:], in0=acc_t[:, D:W136],
                                    scalar1=SEPS, scalar2=None, op0=AL.add)
            nc.vector.reciprocal(out=rs_t[:], in_=rs_t[:])
            he_t = epi.tile([P, P], f32, tag="he")
            nc.gpsimd.tensor_tensor(
                out=he_t[:].rearrange("p (h d) -> p h d", h=H),
                in0=acc_t[:, 0:D].rearrange("p (h d) -> p h d", h=H),
                in1=rs_t[:].unsqueeze(-1).to_broadcast([P, H, DH]),
                op=AL.mult)

            # beta = sigmoid(he.wbh + x.wbx)
            z_t = sml.tile([P, 4], f32, tag="z")
            scr1 = epi.tile([P, P], f32, tag="scr1")
            nc.gpsimd.scalar_tensor_tensor(
                out=scr1[:], in0=he_t[:], scalar=1.0, in1=wbh[:],
                op0=AL.mult, op1=AL.mult, accum_out=z_t[:, 0:1])
            scr2 = epi.tile([P, P], f32, tag="scr2")
            nc.gpsimd.scalar_tensor_tensor(
                out=scr2[:], in0=nxs, scalar=1.0, in1=wbx[:],
                op0=AL.mult, op1=AL.mult, accum_out=z_t[:, 1:2])
            nc.gpsimd.tensor_tensor(out=z_t[:, 2:3], in0=z_t[:, 0:1],
                                    in1=z_t[:, 1:2], op=AL.add)
            beta_t = sml.tile([P, 1], f32, tag="beta")
            nc.scalar.activation(out=beta_t[:], in_=z_t[:, 2:3], func=AF.Exp,
                                 scale=-1.0)
            nc.gpsimd.tensor_scalar(out=beta_t[:], in0=beta_t[:], scalar1=1.0,
                                    scalar2=None, op0=AL.add)
            nc.vector.reciprocal(out=beta_t[:], in_=beta_t[:])

            # he2 = he + beta*(x - he)
            d_t = epi.tile([P, P], f32, tag="d")
            nc.gpsimd.tensor_tensor(out=d_t[:], in0=nxs, in1=he_t[:],
                                    op=AL.subtract)
            he2_t = epi.tile([P, P], f32, tag="he2")
            nc.gpsimd.scalar_tensor_tensor(
                out=he2_t[:], in0=d_t[:], scalar=beta_t[:, 0:1], in1=he_t[:],
                op0=AL.mult, op1=AL.add)

            def layer_norm(src_t, dst_tag, center_eng):
                """dst = LN(src) for one block; returns dst tile."""
                negmu = sml.tile([P, 1], f32, tag="negmu")
                nc.vector.tensor_reduce(out=negmu[:], in_=src_t[:],
                                        axis=mybir.AxisListType.X, op=AL.add,
                                        negate=True)
                nc.vector.tensor_scalar(out=negmu[:], in0=negmu[:],
                                        scalar1=1.0 / P, scalar2=None,
                                        op0=AL.mult)
                hc_t = epi.tile([P, P], f32, tag=dst_tag + "hc")
                nc.gpsimd.tensor_tensor(out=hc_t[:], in0=src_t[:],
                                        in1=negmu[:, 0:1].to_broadcast([P, P]),
                                        op=AL.add)
                sq_t = epi.tile([P, P], f32, tag=dst_tag + "sq")
                var_t = sml.tile([P, 1], f32, tag="var")
                nc.gpsimd.scalar_tensor_tensor(
                    out=sq_t[:], in0=hc_t[:], scalar=1.0, in1=hc_t[:],
                    op0=AL.mult, op1=AL.mult, accum_out=var_t[:])
                nc.gpsimd.tensor_scalar(out=var_t[:], in0=var_t[:],
                                        scalar1=1.0 / P, scalar2=EPS,
                                        op0=AL.mult, op1=AL.add)
                nc.scalar.activation(out=var_t[:], in_=var_t[:], func=AF.Ln)
                nc.scalar.activation(out=var_t[:], in_=var_t[:], func=AF.Exp,
                                     scale=-0.5)
                y_t = epi.tile([P, P], f32, tag=dst_tag)
                nc.vector.tensor_scalar(out=y_t[:], in0=hc_t[:],
                                        scalar1=var_t[:, 0:1], scalar2=None,
                                        op0=AL.mult)
                return y_t

            y_t = layer_norm(he2_t, "y", None)

            yT_ps = psFFN.tile([P, P], f32, tag="ffn")
            nc.tensor.transpose(out=yT_ps[:], in_=y_t[:], identity=ident[:])
            yT_sb = epi.tile([P, P], bf16, tag="yts")
            nc.scalar.copy(out=yT_sb[:], in_=yT_ps[:])
            h1a_ps = psFFN.tile([P, P], f32, tag="ffn")
            nc.tensor.matmul(out=h1a_ps[:], lhsT=W1T[:, 0:P], rhs=yT_sb[:],
                             start=True, stop=True)
            h1b_ps = psFFN.tile([P, P], f32, tag="ffn")
            nc.tensor.matmul(out=h1b_ps[:], lhsT=W1T[:, P:2 * P], rhs=yT_sb[:],
                             start=True, stop=True)
            h1a_sb = epi.tile([P, P], bf16, tag="h1as")
            nc.scalar.activation(out=h1a_sb[:], in_=h1a_ps[:], func=AF.Relu)
            h1b_sb = epi.tile([P, P], bf16, tag="h1bs")
            nc.scalar.activation(out=h1b_sb[:], in_=h1b_ps[:], func=AF.Relu)
            h2_ps = psFFN.tile([P, P], f32, tag="ffn")
            nc.tensor.matmul(out=h2_ps[:], lhsT=h1a_sb[:], rhs=W2Ta[:],
                             start=True, stop=False)
            nc.tensor.matmul(out=h2_ps[:], lhsT=h1b_sb[:], rhs=W2Tb[:],
                             start=False, stop=True)
            o_t = epi.tile([P, P], f32, tag="o")
            nc.vector.tensor_tensor(out=o_t[:], in0=h2_ps[:], in1=he2_t[:],
                                    op=AL.add)

            out_t = layer_norm(o_t, "out", None)
            nc.sync.dma_start(out=out_dram[:, j * P:(j + 1) * P], in_=out_t[:])

            off += T

    nc.finalize()
    return nc


# --------------------------------------------------------------------------
# entry point
# --------------------------------------------------------------------------

def kernel(**inputs):
    import os
    T_sched, chunks, TT, in_maps, block_orders = _host_prep(inputs)

    key = tuple(T_sched)
    if key not in _CACHE:
        _CACHE[key] = build_program(T_sched, chunks, TT)
    nc = _CACHE[key]

    trace = bool(os.environ.get("BASS_KERNEL_TRACE"))
    tmpdir = os.environ.get("BASS_KERNEL_TRACE_DIR") or None
    results = run_bass_kernel_spmd(nc, in_maps, core_ids=list(range(NCORES)),
                                   trace=trace, tmpdir=tmpdir)
    if trace and results.exec_time_ns is not None:
        print(f"HW exec time: {results.exec_time_ns} ns")

    out = np.zeros((N_NODES, D), np.float32)
    for c in range(NCORES):
        o = results.results[c]["out"]          # [128, NBLK*128]
        base = c * NPC
        for j, bj in enumerate(block_orders[c]):
            lo = base + bj * P
            hi = min(lo + P, base + NPC)
            n = hi - lo
            out[lo:hi, :] = o[:n, j * P:j * P + P]
    return out


# revision 19
# speedup vs baseline: 1.6415x; 1.0805x over previous
"""Trainium2 Bass kernel for nn_Bond2AtomLayer (GNN message passing).

Strategy (8-core SPMD, dst-node partitioned):
- Host: sort edges by dst, partition nodes into 8 ranges of 6250; each core
  owns the edges whose dst falls in its range. Within a core, edges are
  grouped into 128-node "blocks" (49 per core), each block's edges padded to
  a whole number of 128-edge tiles. Blocks are assigned to fixed program
  positions with a global per-position tile count (max over cores) so all
  8 cores run one identical program.
- Host pre-gathers node_emb[src], node_emb[dst] and bond rows into per-core
  edge-order streams, transposed to [feat, edge] (bf16) so the device needs
  no gather at all — k/q/v are computed per 128-edge tile by PE matmuls
  against the small weight matrices.
- Edge softmax: att[e,h] = sum_d k[e,hd]*q[e,hd] (DVE mult + grouped reduce),
  logits = att/4 + ba*W_dis (1/4 folded into Wq), p = exp(logits) without
  max-subtraction (logits are O(1); softmax is shift-invariant so this
  matches the reference numerically).
- Scatter-sum: per tile a one-hot S[e,n] (bf16, built by GPSIMD local_scatter)
  and one PE matmul accumulates [wv | p] into PSUM per 128-node block;
  ft = wsum / s, then beta-gating, LN, FFN, LN on-chip (node-major, batched).
"""
import sys

sys.path.insert(0, "/opt/trn_rl_repo")

import numpy as np
import ml_dtypes
from contextlib import ExitStack

import concourse.bass as bass
import concourse.tile as tile
from concourse import bacc, mybir
from concourse.bass_utils import run_bass_kernel_spmd

BF16 = ml_dtypes.bfloat16

N_NODES = 50000
N_EDGES = 800000
D = 128
H = 8
DH = 16
D_FF = 256
P = 128
NCORES = 8
NPC = N_NODES // NCORES        # 6250 nodes per core
NBLK = (NPC + P - 1) // P      # 49 blocks per core (last has 106 nodes)
NPAD = NBLK * P                # 6272
EPS = 1e-5
SEPS = 1e-30                   # guard for 1/s on isolated nodes
W136 = D + H                   # 136: [wv | p] scatter payload width

_CACHE = {}


# --------------------------------------------------------------------------
# host-side scheduling
# --------------------------------------------------------------------------

def _schedule(dst):
    """Partition edges by dst; build per-core block schedules.

    Returns:
      T_sched: list of per-position tile counts (same for all cores)
      chunks:  list of chunk widths per position (even, <=4 each)
      per_core: list of dicts with keys:
        edge_perm [TT*128] int64 (index into full edge list; -1 = pad)
        dst_local [TT*128] int16 (node index within block; -1 = pad)
        block_order: list of per-position original block ids
    """
    E = dst.shape[0]
    core = dst // NPC
    blk = (dst % NPC) // P
    nloc = (dst % NPC) % P

    order = np.argsort(dst, kind="stable")

    per_core_raw = []
    counts = np.zeros((NCORES, NBLK), np.int64)
    for c in range(NCORES):
        sel = order[(core[order] == c)]
        b = blk[sel]
        blists = []
        for j in range(NBLK):
            eb = sel[b == j]
            blists.append(eb)
            counts[c, j] = len(eb)
        per_core_raw.append(blists)

    tiles = (counts + P - 1) // P           # [NCORES, NBLK]
    tiles = np.maximum(tiles, 1)
    # sort each core's blocks by tile count desc; per-position count = max
    orders = [list(np.argsort(-tiles[c], kind="stable")) for c in range(NCORES)]
    T_sched = []
    for j in range(NBLK):
        t = max(tiles[c, orders[c][j]] for c in range(NCORES))
        t = int(t + (t & 1))                # round up to even
        T_sched.append(max(t, 2))

    chunks = []
    for t in T_sched:
        ch = [4] * (t // 4)
        if t % 4:
            ch.append(t % 4)                # t even => remainder 2
        chunks.append(ch)

    TT = sum(T_sched)
    per_core = []
    for c in range(NCORES):
        perm = np.full(TT * P, -1, np.int64)
        dloc = np.full(TT * P, -1, np.int16)
        off = 0
        for j in range(NBLK):
            bj = orders[c][j]
            eb = per_core_raw[c][bj]
            perm[off:off + len(eb)] = eb
            dloc[off:off + len(eb)] = nloc[eb].astype(np.int16)
            off += T_sched[j] * P
        per_core.append(dict(edge_perm=perm, dst_local=dloc, block_order=orders[c]))
    return T_sched, chunks, TT, per_core


def _host_prep(inputs):
    bond = np.asarray(inputs["bond_embedding"], np.float32)
    nemb = np.asarray(inputs["node_embedding"], np.float32)
    ba = np.asarray(inputs["basic_attn"], np.float32).reshape(-1)
    Wk = np.asarray(inputs["Wk"], np.float32)
    Wq = np.asarray(inputs["Wq"], np.float32)
    Wv = np.asarray(inputs["Wv"], np.float32)
    W_dis = np.asarray(inputs["W_dis"], np.float32)
    W_beta = np.asarray(inputs["W_beta"], np.float32).reshape(-1)
    W1 = np.asarray(inputs["W1"], np.float32)
    W2 = np.asarray(inputs["W2"], np.float32)
    src = np.asarray(inputs["src"], np.int64)
    dst = np.asarray(inputs["dst"], np.int64)

    T_sched, chunks, TT, per_core = _schedule(dst)

    # weights (replicated)
    scale = 1.0 / np.sqrt(np.float32(DH))
    consts = dict(
        WkT=np.ascontiguousarray(Wk.T).astype(BF16),
        WqT=np.ascontiguousarray((Wq * scale).T).astype(BF16),
        WvT=np.ascontiguousarray(Wv.T).astype(BF16),
        W1T=np.ascontiguousarray(W1.T).astype(BF16),        # [128, 256]
        W2T=np.ascontiguousarray(W2.T).astype(BF16),        # [256, 128]
        wdis=np.broadcast_to(W_dis.reshape(1, H), (P, H)).astype(np.float32).copy(),
        wbh=np.broadcast_to((W_beta[0:D] + W_beta[2 * D:3 * D]).reshape(1, D), (P, D)).astype(np.float32).copy(),
        wbx=np.broadcast_to((W_beta[D:2 * D] - W_beta[2 * D:3 * D]).reshape(1, D), (P, D)).astype(np.float32).copy(),
        ident=np.eye(P, dtype=np.float32),
        ones4=np.ones((P, 4), BF16),
    )

    nembT = np.ascontiguousarray(nemb.T)    # [128, N]

    in_maps = []
    unperm = []
    for c in range(NCORES):
        pc = per_core[c]
        perm = pc["edge_perm"]
        safe = np.where(perm >= 0, perm, 0)

        # streams in [feat, edge] layout, bf16
        embsT = np.ascontiguousarray(nembT[:, src[safe]]).astype(BF16)
        embdT = np.ascontiguousarray(nembT[:, dst[safe]]).astype(BF16)
        bondT = np.ascontiguousarray(bond[safe].T).astype(BF16)

        # per-tile-partition arrays [128, TT]: element (p, t) = edge t*128+p
        ba_pm = np.ascontiguousarray(
            np.where(perm >= 0, ba[safe], 0.0).astype(np.float32).reshape(TT, P).T)
        dloc = pc["dst_local"].astype(np.int32).reshape(TT, P).T  # [128, TT]
        # local_scatter indices: within chunk, column = tile_in_chunk*128 + dst_local
        sidx = np.full((P, TT), -1, np.int32)
        off = 0
        for j in range(NBLK):
            for ch in chunks[j]:
                for t in range(ch):
                    col = off + t
                    d_ = dloc[:, col]
                    sidx[:, col] = np.where(d_ >= 0, t * P + d_, -1)
                off += ch
        sidx = sidx.astype(np.int16)

        # node-side: local x in [128, NBLK*128] partition-major by block,
        # following block_order (position j holds original block order[j])
        nx = np.zeros((P, NBLK * P), np.float32)
        base = c * NPC
        for j, bj in enumerate(pc["block_order"]):
            lo = base + bj * P
            hi = min(lo + P, base + NPC)
            n = hi - lo
            nx[:n, j * P:j * P + P] = nemb[lo:hi, :]

        in_maps.append(dict(
            embsT=embsT, embdT=embdT, bondT=bondT,
            ba=ba_pm, sidx=sidx, nx=nx, **consts))
        unperm.append(pc["block_order"])

    return T_sched, chunks, TT, in_maps, unperm


# --------------------------------------------------------------------------
# device program
# --------------------------------------------------------------------------

def build_program(T_sched, chunks, TT):
    # Pin a single ACT function table covering every func we use (exp, ln,
    # copy, relu, identity, square) so no mid-kernel table reloads (~1.3us
    # each) are emitted.
    _orig_tables = bacc.get_activation_tables
    def _one_table(arch):
        tabs = _orig_tables(arch)
        name = "natural_log_exp_and_others"
        return {name: tabs[name]} if name in tabs else tabs
    bacc.get_activation_tables = _one_table
    try:
        return _build_program_inner(T_sched, chunks, TT)
    finally:
        bacc.get_activation_tables = _orig_tables


def _build_program_inner(T_sched, chunks, TT):
    nc = bacc.Bacc("TRN2", target_bir_lowering=False, debug=False,
                   num_devices=NCORES)
    f32 = mybir.dt.float32
    bf16 = mybir.dt.bfloat16
    i16 = mybir.dt.int16
    AL = mybir.AluOpType
    AF = mybir.ActivationFunctionType

    embsT_in = nc.dram_tensor("embsT", [P, TT * P], bf16, kind="ExternalInput")
    embdT_in = nc.dram_tensor("embdT", [P, TT * P], bf16, kind="ExternalInput")
    bondT_in = nc.dram_tensor("bondT", [P, TT * P], bf16, kind="ExternalInput")
    ba_in = nc.dram_tensor("ba", [P, TT], f32, kind="ExternalInput")
    sidx_in = nc.dram_tensor("sidx", [P, TT], i16, kind="ExternalInput")
    nx_in = nc.dram_tensor("nx", [P, NBLK * P], f32, kind="ExternalInput")
    WkT_in = nc.dram_tensor("WkT", [P, P], bf16, kind="ExternalInput")
    WqT_in = nc.dram_tensor("WqT", [P, P], bf16, kind="ExternalInput")
    WvT_in = nc.dram_tensor("WvT", [P, P], bf16, kind="ExternalInput")
    W1T_in = nc.dram_tensor("W1T", [P, D_FF], bf16, kind="ExternalInput")
    W2T_in = nc.dram_tensor("W2T", [D_FF, P], bf16, kind="ExternalInput")
    wdis_in = nc.dram_tensor("wdis", [P, H], f32, kind="ExternalInput")
    wbh_in = nc.dram_tensor("wbh", [P, D], f32, kind="ExternalInput")
    wbx_in = nc.dram_tensor("wbx", [P, D], f32, kind="ExternalInput")
    ident_in = nc.dram_tensor("ident", [P, P], f32, kind="ExternalInput")
    ones4_in = nc.dram_tensor("ones4", [P, 4], bf16, kind="ExternalInput")

    out_dram = nc.dram_tensor("out", [P, NBLK * P], f32, kind="ExternalOutput")

    with ExitStack() as ctx:
        tc = ctx.enter_context(tile.TileContext(nc))
        cst = ctx.enter_context(tc.tile_pool(name="cst", bufs=1))
        res = ctx.enter_context(tc.tile_pool(name="res", bufs=1))
        edg = ctx.enter_context(tc.tile_pool(name="edg", bufs=4))
        sml = ctx.enter_context(tc.tile_pool(name="sml", bufs=4))
        wrk = ctx.enter_context(tc.tile_pool(name="wrk", bufs=3))
        epi = ctx.enter_context(tc.tile_pool(name="epi", bufs=3))
        psMM = ctx.enter_context(tc.tile_pool(name="psMM", bufs=5, space="PSUM"))
        psFFN = ctx.enter_context(tc.tile_pool(name="psFFN", bufs=1, space="PSUM"))
        psACC = ctx.enter_context(tc.tile_pool(name="psACC", bufs=2, space="PSUM"))

        def load_const(inp, shape, dtype, tag):
            t = cst.tile(shape, dtype, tag=tag)
            nc.sync.dma_start(out=t[:], in_=inp[:, :])
            return t

        WkT = load_const(WkT_in, [P, P], bf16, "WkT")
        WqT = load_const(WqT_in, [P, P], bf16, "WqT")
        WvT = load_const(WvT_in, [P, P], bf16, "WvT")
        W1T = load_const(W1T_in, [P, D_FF], bf16, "W1T")
        wdis = load_const(wdis_in, [P, H], f32, "wdis")
        wbh = load_const(wbh_in, [P, D], f32, "wbh")
        wbx = load_const(wbx_in, [P, D], f32, "wbx")
        ident = load_const(ident_in, [P, P], f32, "ident")
        ones4 = load_const(ones4_in, [P, 4], bf16, "ones4")

        eps_t = cst.tile([P, 1], f32, tag="eps")
        nc.gpsimd.memset(eps_t[:], EPS)

        W2Ta = cst.tile([P, P], bf16, tag="W2Ta")
        nc.sync.dma_start(out=W2Ta[:], in_=W2T_in[0:P, :])
        W2Tb = cst.tile([P, P], bf16, tag="W2Tb")
        nc.sync.dma_start(out=W2Tb[:], in_=W2T_in[P:2 * P, :])

        nx_res = res.tile([P, NBLK * P], f32)
        nc.sync.dma_start(out=nx_res[:], in_=nx_in[:, :])

        off = 0
        for j in range(NBLK):
            T = T_sched[j]
            e0 = off * P

            bond_t = edg.tile([P, T * P], bf16, tag="bond")
            nc.sync.dma_start(out=bond_t[:], in_=bondT_in[:, e0:e0 + T * P])
            embs_t = edg.tile([P, T * P], bf16, tag="embs")
            nc.sync.dma_start(out=embs_t[:], in_=embsT_in[:, e0:e0 + T * P])
            embd_t = edg.tile([P, T * P], bf16, tag="embd")
            nc.sync.dma_start(out=embd_t[:], in_=embdT_in[:, e0:e0 + T * P])
            ba_t = sml.tile([P, T], f32, tag="ba")
            nc.sync.dma_start(out=ba_t[:], in_=ba_in[:, off:off + T])
            sidx_t = sml.tile([P, T], i16, tag="sidx")
            nc.sync.dma_start(out=sidx_t[:], in_=sidx_in[:, off:off + T])

            att_t = sml.tile([P, T * H], f32, tag="att")
            wvp_t = wrk.tile([P, T * W136], bf16, tag="wvp")

            # bias = ba*wdis (no dep on att; runs early on Pool)
            bias_t = sml.tile([P, T * H], f32, tag="bias")
            nc.gpsimd.tensor_tensor(
                out=bias_t[:].rearrange("p (t h) -> p t h", h=H),
                in0=ba_t[:].unsqueeze(-1).to_broadcast([P, T, H]),
                in1=wdis[:].unsqueeze(1).to_broadcast([P, T, H]),
                op=AL.mult)

            # pass 1: attention logits
            c0 = 0
            for cw in chunks[j]:
                q_ps = psMM.tile([P, cw * P], f32, tag="mm")
                for t in range(cw):
                    nc.tensor.matmul(
                        out=q_ps[:, t * P:(t + 1) * P],
                        lhsT=embd_t[:, (c0 + t) * P:(c0 + t + 1) * P],
                        rhs=WqT[:], start=True, stop=True)
                q_sb = wrk.tile([P, cw * P], bf16, tag="qsb")
                nc.scalar.copy(out=q_sb[:], in_=q_ps[:])
                k_ps = psMM.tile([P, cw * P], f32, tag="mm")
                for t in range(cw):
                    nc.tensor.matmul(
                        out=k_ps[:, t * P:(t + 1) * P],
                        lhsT=embs_t[:, (c0 + t) * P:(c0 + t + 1) * P],
                        rhs=WkT[:], start=True, stop=True)
                k_sb = wrk.tile([P, cw * P], bf16, tag="ksb")
                nc.scalar.copy(out=k_sb[:], in_=k_ps[:])
                kq_t = wrk.tile([P, cw * P], bf16, tag="kq")
                nc.vector.tensor_tensor(out=kq_t[:], in0=k_sb[:], in1=q_sb[:],
                                        op=AL.mult)
                nc.vector.tensor_reduce(
                    out=att_t[:].rearrange("p (t h) -> p t h", h=H)[:, c0:c0 + cw, :],
                    in_=kq_t[:].rearrange("p (t h d) -> p t h d", h=H, d=DH),
                    axis=mybir.AxisListType.X, op=AL.add)
                c0 += cw

            # logits -> p, written into the p-slots of wvp
            nc.vector.tensor_tensor(out=bias_t[:], in0=bias_t[:], in1=att_t[:],
                                    op=AL.add)
            p_t = sml.tile([P, T * H], bf16, tag="pexp")
            nc.scalar.activation(out=p_t[:], in_=bias_t[:], func=AF.Exp)
            nc.vector.tensor_copy(
                out=wvp_t[:].rearrange("p (t w) -> p t w", w=W136)[:, :, D:W136],
                in_=p_t[:].rearrange("p (t h) -> p t h", h=H))

            # pass 2: v, wv, one-hot scatter
            acc_ps = psACC.tile([P, 512], f32, tag="acc")
            c0 = 0
            for cw in chunks[j]:
                v_ps = psMM.tile([P, cw * P], f32, tag="mm")
                for t in range(cw):
                    nc.tensor.matmul(
                        out=v_ps[:, t * P:(t + 1) * P],
                        lhsT=bond_t[:, (c0 + t) * P:(c0 + t + 1) * P],
                        rhs=WvT[:], start=True, stop=True)
                s_t = wrk.tile([P, cw * P], bf16, tag="sh")
                nc.gpsimd.local_scatter(
                    out_ap=s_t[:], data_ap=ones4[:, 0:cw],
                    idxs_ap=sidx_t[:, c0:c0 + cw],
                    channels=P, num_elems=cw * P, num_idxs=cw)
                nc.vector.tensor_tensor(
                    out=wvp_t[:].rearrange("p (t w) -> p t w", w=W136)
                        [:, c0:c0 + cw, 0:D].rearrange("p t (h d) -> p t h d", h=H),
                    in0=v_ps[:].rearrange("p (t h d) -> p t h d", h=H, d=DH),
                    in1=wvp_t[:].rearrange("p (t w) -> p t w", w=W136)
                        [:, c0:c0 + cw, D:W136].unsqueeze(-1).to_broadcast([P, cw, H, DH]),
                    op=AL.mult)
                for t in range(cw):
                    gt = c0 + t
                    nc.tensor.matmul(
                        out=acc_ps[:, 0:W136],
                        lhsT=s_t[:, t * P:(t + 1) * P],
                        rhs=wvp_t[:, gt * W136:(gt + 1) * W136],
                        start=(gt == 0), stop=(gt == T - 1))
                c0 += cw

            # ---- per-block node epilogue (overlaps with later blocks) ----
            nxs = nx_res[:, j * P:(j + 1) * P]
            acc_t = epi.tile([P, W136], f32, tag="acct")
            nc.scalar.copy(out=acc_t[:], in_=acc_ps[:, 0:W136])

            rs_t = sml.tile([P, H], f32, tag="rs")
            nc.gpsimd.tensor_scalar(out=rs_t[:], in0=acc_t[:, D:W136],
                                    scalar1=SEPS, scalar2=None, op0=AL.add)
            nc.vector.reciprocal(out=rs_t[:], in_=rs_t[:])
            he_t = epi.tile([P, P], f32, tag="he")
            nc.gpsimd.tensor_tensor(
                out=he_t[:].rearrange("p (h d) -> p h d", h=H),
                in0=acc_t[:, 0:D].rearrange("p (h d) -> p h d", h=H),
                in1=rs_t[:].unsqueeze(-1).to_broadcast([P, H, DH]),
                op=AL.mult)

            # beta = sigmoid(he.wbh + x.wbx)
            z_t = sml.tile([P, 4], f32, tag="z")
            scr1 = epi.tile([P, P], f32, tag="scr1")
            nc.gpsimd.scalar_tensor_tensor(
                out=scr1[:], in0=he_t[:], scalar=1.0, in1=wbh[:],
                op0=AL.mult, op1=AL.mult, accum_out=z_t[:, 0:1])
            scr2 = epi.tile([P, P], f32, tag="scr2")
            nc.gpsimd.scalar_tensor_tensor(
                out=scr2[:], in0=nxs, scalar=1.0, in1=wbx[:],
                op0=AL.mult, op1=AL.mult, accum_out=z_t[:, 1:2])
            nc.gpsimd.tensor_tensor(out=z_t[:, 2:3], in0=z_t[:, 0:1],
                                    in1=z_t[:, 1:2], op=AL.add)
            beta_t = sml.tile([P, 1], f32, tag="beta")
            nc.scalar.activation(out=beta_t[:], in_=z_t[:, 2:3], func=AF.Exp,
                                 scale=-1.0)
            nc.gpsimd.tensor_scalar(out=beta_t[:], in0=beta_t[:], scalar1=1.0,
                                    scalar2=None, op0=AL.add)
            nc.vector.reciprocal(out=beta_t[:], in_=beta_t[:])

            # he2 = he + beta*(x - he)
            d_t = epi.tile([P, P], f32, tag="d")
            nc.gpsimd.tensor_tensor(out=d_t[:], in0=nxs, in1=he_t[:],
                                    op=AL.subtract)
            he2_t = epi.tile([P, P], f32, tag="he2")
            nc.gpsimd.scalar_tensor_tensor(
                out=he2_t[:], in0=d_t[:], scalar=beta_t[:, 0:1], in1=he_t[:],
                op0=AL.mult, op1=AL.add)

            def layer_norm(src_t, dst_tag, center_eng):
                """dst = LN(src) for one block; returns dst tile."""
                negmu = sml.tile([P, 1], f32, tag="negmu")
                nc.vector.tensor_reduce(out=negmu[:], in_=src_t[:],
                                        axis=mybir.AxisListType.X, op=AL.add,
                                        negate=True)
                nc.vector.tensor_scalar(out=negmu[:], in0=negmu[:],
                                        scalar1=1.0 / P, scalar2=None,
                                        op0=AL.mult)
                hc_t = epi.tile([P, P], f32, tag=dst_tag + "hc")
                nc.gpsimd.tensor_tensor(out=hc_t[:], in0=src_t[:],
                                        in1=negmu[:, 0:1].to_broadcast([P, P]),
                                        op=AL.add)
                sq_t = epi.tile([P, P], f32, tag=dst_tag + "sq")
                var_t = sml.tile([P, 1], f32, tag="var")
                nc.gpsimd.scalar_tensor_tensor(
                    out=sq_t[:], in0=hc_t[:], scalar=1.0, in1=hc_t[:],
                    op0=AL.mult, op1=AL.mult, accum_out=var_t[:])
                nc.gpsimd.tensor_scalar(out=var_t[:], in0=var_t[:],
                                        scalar1=1.0 / P, scalar2=EPS,
                                        op0=AL.mult, op1=AL.add)
                nc.scalar.activation(out=var_t[:], in_=var_t[:], func=AF.Ln)
                nc.scalar.activation(out=var_t[:], in_=var_t[:], func=AF.Exp,
                                     scale=-0.5)
                y_t = epi.tile([P, P], f32, tag=dst_tag)
                nc.vector.tensor_scalar(out=y_t[:], in0=hc_t[:],
                                        scalar1=var_t[:, 0:1], scalar2=None,
                                        op0=AL.mult)
                return y_t

            y_t = layer_norm(he2_t, "y", None)

            yT_ps = psFFN.tile([P, P], f32, tag="ffn")
            nc.tensor.transpose(out=yT_ps[:], in_=y_t[:], identity=ident[:])
            yT_sb = epi.tile([P, P], bf16, tag="yts")
            nc.scalar.copy(out=yT_sb[:], in_=yT_ps[:])
            h1a_ps = psFFN.tile([P, P], f32, tag="ffn")
            nc.tensor.matmul(out=h1a_ps[:], lhsT=W1T[:, 0:P], rhs=yT_sb[:],
                             start=True, stop=True)
            h1b_ps = psFFN.tile([P, P], f32, tag="ffn")
            nc.tensor.matmul(out=h1b_ps[:], lhsT=W1T[:, P:2 * P], rhs=yT_sb[:],
                             start=True, stop=True)
            h1a_sb = epi.tile([P, P], bf16, tag="h1as")
            nc.scalar.activation(out=h1a_sb[:], in_=h1a_ps[:], func=AF.Relu)
            h1b_sb = epi.tile([P, P], bf16, tag="h1bs")
            nc.scalar.activation(out=h1b_sb[:], in_=h1b_ps[:], func=AF.Relu)
            h2_ps = psFFN.tile([P, P], f32, tag="ffn")
            nc.tensor.matmul(out=h2_ps[:], lhsT=h1a_sb[:], rhs=W2Ta[:],
                             start=True, stop=False)
            nc.tensor.matmul(out=h2_ps[:], lhsT=h1b_sb[:], rhs=W2Tb[:],
                             start=False, stop=True)
            o_t = epi.tile([P, P], f32, tag="o")
            nc.vector.tensor_tensor(out=o_t[:], in0=h2_ps[:], in1=he2_t[:],
                                    op=AL.add)

            out_t = layer_norm(o_t, "out", None)
            nc.sync.dma_start(out=out_dram[:, j * P:(j + 1) * P], in_=out_t[:])

            off += T

    nc.finalize()
    return nc


# --------------------------------------------------------------------------
# entry point
# --------------------------------------------------------------------------

def kernel(**inputs):
    import os
    T_sched, chunks, TT, in_maps, block_orders = _host_prep(inputs)

    key = tuple(T_sched)
    if key not in _CACHE:
        _CACHE[key] = build_program(T_sched, chunks, TT)
    nc = _CACHE[key]

    trace = bool(os.environ.get("BASS_KERNEL_TRACE"))
    tmpdir = os.environ.get("BASS_KERNEL_TRACE_DIR") or None
    results = run_bass_kernel_spmd(nc, in_maps, core_ids=list(range(NCORES)),
                                   trace=trace, tmpdir=tmpdir)
    if trace and results.exec_time_ns is not None:
        print(f"HW exec time: {results.exec_time_ns} ns")

    out = np.zeros((N_NODES, D), np.float32)
    for c in range(NCORES):
        o = results.results[c]["out"]          # [128, NBLK*128]
        base = c * NPC
        for j, bj in enumerate(block_orders[c]):
            lo = base + bj * P
            hi = min(lo + P, base + NPC)
            n = hi - lo
            out[lo:hi, :] = o[:n, j * P:j * P + P]
    return out
